# revision 16
# baseline (speedup 1.0000x reference)
"""Trainium2 Bass kernel for CorrespondenceGenerationArch.

Per-core (8 cores = 2 samples x 4 quarters):
  - per-pixel L2 feature normalization (on device)
  - correlation GEMM [1664 x 2304] @ [2304 x 6240] (fp32, PE)
  - top-8 values+indices per output row (DVE max/max_index)
  - VGG head (conv1_1..conv3_1) on 2 spatial strips
Host: input slicing/padding, weight folding, flow/offset/sim expansion from
the top-3 indices/values (pure index arithmetic), output assembly.
"""
import sys

if "/opt/trn_rl_repo" not in sys.path:
    sys.path.insert(0, "/opt/trn_rl_repo")

import numpy as np

import concourse.bass as bass
import concourse.mybir as mybir
import concourse.tile as tile
from concourse.bass_utils import run_bass_kernel_spmd

F32 = mybir.dt.float32
U32 = mybir.dt.uint32
AF = mybir.ActivationFunctionType

# ---------------- problem constants (hardcoded) ----------------
B, C, H, W = 2, 256, 80, 80
PIX = H * W                      # 6400
W2 = H - 2                       # 78
NREF = W2 * W                    # 6240 grid columns (i<78, all j)
NT, NTW = 13, 480                # N tiles: 13 x 480 = 6240
MC = 13                          # M chunks of 128 per core
MPC = MC * 128                   # 1664 M rows per core
FINW = 1856                      # fin slice width (1664 + 162 pad -> 1856)
FREFW = 6592                     # fref width (6400 + 162 pad -> 6592)
DKS = [di * W + dj for di in range(3) for dj in range(3)]  # patch offsets
EPS_PATCH = 1e-5

# VGG strip geometry (per job; 16 strips of 40 rows at 320-res, 2 jobs/core)
JOBS = 2
W322, W328 = 322, 328
C11R = 60          # c11 buffer rows (58 valid + slack)
CR11 = 10          # conv1_1 chunk rows
CR12 = 12          # conv1_2 chunk rows (must be even)
P1R, P1W = 30, 162   # pool1 buffer rows/width (28 valid + 2)
C21R = 28          # conv2_1 buffer rows (26 valid + 2)
C22R = 24          # conv2_2 rows (exactly valid)
P2R, P2W = 14, 82    # pool2 rows/width (12 valid + 2)

_CACHE = {}


# ---------------- BIR post-pass: walrus accepts 1 sync-wait/instruction ----
def _split_excess_waits(nc, cap=1):
    n = 0
    for f in nc.m.functions:
        for bb in f.blocks:
            il = bb.instructions
            out = []
            changed = False
            for ins in il:
                si = getattr(ins, "sync_info", None)
                ow = list(si.on_wait) if si is not None and si.on_wait else []
                k = 0
                while len(ow) > cap:
                    chunk, ow = ow[:cap], ow[cap:]
                    out.append(mybir.InstNoOp(
                        name=f"{ins.name}_ws{k}",
                        sync_info=mybir.SyncInfo(on_wait=chunk, on_update=[]),
                        engine=ins.engine,
                        bass_nofuse=True,
                    ))
                    k += 1
                    n += 1
                if k:
                    si.on_wait = ow
                    changed = True
                out.append(ins)
            if changed:
                il[:] = out
    return n


# ---------------- device program ----------------
def _normalize(nc, tc, pool, psum, f_t, width, subtiles, ones_col, ones_row):
    """Per-pixel (column) L2-normalize f_t[2][128, width] in place."""
    row_pool = pool
    sums = row_pool.tile([1, width], F32, tag="normrow")
    for s0, sw in subtiles:
        sq = row_pool.tile([128, NTW], F32, tag="sqtmp")
        ps = psum.tile([1, NTW], F32, tag="normps")
        for ch in range(2):
            nc.vector.tensor_mul(sq[:, :sw], f_t[ch][:, s0:s0 + sw], f_t[ch][:, s0:s0 + sw])
            nc.tensor.matmul(ps[:, :sw], ones_col[:], sq[:, :sw],
                             start=(ch == 0), stop=(ch == 1))
        nc.vector.tensor_copy(sums[:, s0:s0 + sw], ps[:, :sw])
    # norm = max(sqrt(sumsq), 1e-12); r = 1/norm
    nc.scalar.activation(sums[:], sums[:], AF.Sqrt)
    nc.vector.tensor_scalar_max(sums[:], sums[:], 1.0e-12)
    nc.vector.reciprocal(sums[:], sums[:])
    for s0, sw in subtiles:
        bc = psum.tile([128, NTW], F32, tag="bcps")
        nc.tensor.matmul(bc[:, :sw], ones_row[:], sums[:, s0:s0 + sw],
                         start=True, stop=True)
        for ch in range(2):
            nc.vector.tensor_mul(f_t[ch][:, s0:s0 + sw], f_t[ch][:, s0:s0 + sw], bc[:, :sw])


def _subtiles(width):
    out = []
    s = 0
    while s < width:
        out.append((s, min(NTW, width - s)))
        s += NTW
    return out


def _build_program():
    nc = bass.Bass()

    fin_d = nc.dram_tensor("fin", [C, FINW], F32, kind="ExternalInput")
    fref_d = nc.dram_tensor("fref", [C, FREFW], F32, kind="ExternalInput")
    imgp_d = nc.dram_tensor("imgp", [JOBS, 3, C11R, W328], F32, kind="ExternalInput")
    w1c_d = nc.dram_tensor("w1c", [27, 64], F32, kind="ExternalInput")
    b1_d = nc.dram_tensor("b1", [64, 1], F32, kind="ExternalInput")
    w12p_d = nc.dram_tensor("w12p", [128, 3, 64], F32, kind="ExternalInput")
    w12l_d = nc.dram_tensor("w12l", [64, 3, 64], F32, kind="ExternalInput")
    b12_d = nc.dram_tensor("b12", [64, 1], F32, kind="ExternalInput")
    w21p_d = nc.dram_tensor("w21p", [128, 3, 128], F32, kind="ExternalInput")
    w21l_d = nc.dram_tensor("w21l", [64, 3, 128], F32, kind="ExternalInput")
    b21_d = nc.dram_tensor("b21", [128, 1], F32, kind="ExternalInput")
    w22_d = nc.dram_tensor("w22", [128, 9, 128], F32, kind="ExternalInput")
    b22_d = nc.dram_tensor("b22", [128, 1], F32, kind="ExternalInput")
    w31_d = nc.dram_tensor("w31", [128, 18, 128], F32, kind="ExternalInput")
    b31_d = nc.dram_tensor("b31", [128, 2], F32, kind="ExternalInput")
    # per-level out-of-image row masks: segments c11[0:60] p1[60:90] c21[90:118] p2[118:132]
    vmask_d = nc.dram_tensor("vmask", [JOBS, 128, 132], F32, kind="ExternalInput")

    topv_d = nc.dram_tensor("topv", [MPC, 8], F32, kind="ExternalOutput")
    topi_d = nc.dram_tensor("topi", [MPC, 8], U32, kind="ExternalOutput")
    r1_d = nc.dram_tensor("r1o", [JOBS, 64, 40, 320], F32, kind="ExternalOutput")
    r2_d = nc.dram_tensor("r2o", [JOBS, 128, 20, 160], F32, kind="ExternalOutput")
    r3_d = nc.dram_tensor("r3o", [JOBS, 2, 128, 10, 80], F32, kind="ExternalOutput")

    with tile.TileContext(nc) as tc:
        # ---------------- phase 1: correlation + topk ----------------
        with tc.tile_pool(name="feat", bufs=1) as feat, \
             tc.tile_pool(name="corr", bufs=2) as corrp, \
             tc.tile_pool(name="small", bufs=2) as small, \
             tc.tile_pool(name="gpsum", bufs=2, space="PSUM") as gpsum:

            ones_col = feat.tile([128, 1], F32)
            nc.vector.memset(ones_col[:], 1.0)
            ones_row = feat.tile([1, 128], F32)
            nc.vector.memset(ones_row[:], 1.0)

            fin_t = [feat.tile([128, FINW], F32, tag=f"fin{ch}", name=f"fin{ch}") for ch in range(2)]
            fref_t = [feat.tile([128, FREFW], F32, tag=f"fref{ch}", name=f"fref{ch}") for ch in range(2)]
            for ch in range(2):
                for s0, sw in _subtiles(FINW):
                    nc.sync.dma_start(fin_t[ch][:, s0:s0 + sw],
                                      fin_d[ch * 128:(ch + 1) * 128, s0:s0 + sw])
                for s0, sw in _subtiles(FREFW):
                    nc.sync.dma_start(fref_t[ch][:, s0:s0 + sw],
                                      fref_d[ch * 128:(ch + 1) * 128, s0:s0 + sw])

            _normalize(nc, tc, small, gpsum, fin_t, FINW, _subtiles(FINW), ones_col, ones_row)
            _normalize(nc, tc, small, gpsum, fref_t, FREFW, _subtiles(FREFW), ones_col, ones_row)

            for mc in range(MC):
                m0 = mc * 128
                corr = corrp.tile([128, NREF], F32, tag="corr")
                for j in range(NT):
                    n0 = j * NTW
                    pt = gpsum.tile([128, NTW], F32, tag="gemm")
                    for kt in range(18):
                        ch, dk = kt // 9, DKS[kt % 9]
                        nc.tensor.matmul(
                            pt[:],
                            fin_t[ch][:, m0 + dk:m0 + dk + 128],
                            fref_t[ch][:, n0 + dk:n0 + dk + NTW],
                            start=(kt == 0), stop=(kt == 17))
                    nc.vector.tensor_copy(corr[:, n0:n0 + NTW], pt[:])
                # mask out grid columns with j >= 78
                cview = corr[:].rearrange("p (r c) -> p r c", c=W)
                nc.vector.memset(cview[:, :, W2:W], -1.0e30)
                tv = small.tile([128, 8], F32, tag="tv")
                ti = small.tile([128, 8], U32, tag="ti")
                nc.vector.max(out=tv[:], in_=corr[:])
                nc.vector.max_index(out=ti[:], in_max=tv[:], in_values=corr[:])
                nc.sync.dma_start(topv_d[m0:m0 + 128, :], tv[:])
                nc.sync.dma_start(topi_d[m0:m0 + 128, :], ti[:])

        # ---------------- phase 2: VGG head ----------------
        with tc.tile_pool(name="vw", bufs=1) as vw, \
             tc.tile_pool(name="vbuf", bufs=1) as vbuf, \
             tc.tile_pool(name="vchunk", bufs=2) as vchunk, \
             tc.tile_pool(name="vpsum", bufs=4, space="PSUM") as vpsum:

            w1c_t = vw.tile([27, 64], F32)
            nc.sync.dma_start(w1c_t[:], w1c_d[:])
            w12p_t = vw.tile([128, 3, 64], F32)
            nc.sync.dma_start(w12p_t[:], w12p_d[:])
            w12l_t = vw.tile([64, 3, 64], F32)
            nc.sync.dma_start(w12l_t[:], w12l_d[:])
            w21p_t = vw.tile([128, 3, 128], F32)
            nc.sync.dma_start(w21p_t[:], w21p_d[:])
            w21l_t = vw.tile([64, 3, 128], F32)
            nc.sync.dma_start(w21l_t[:], w21l_d[:])
            w22_t = vw.tile([128, 9, 128], F32)
            nc.sync.dma_start(w22_t[:], w22_d[:])
            w31_t = vw.tile([128, 18, 128], F32)
            nc.sync.dma_start(w31_t[:], w31_d[:])
            b1_t = vw.tile([64, 1], F32)
            nc.sync.dma_start(b1_t[:], b1_d[:])
            b12_t = vw.tile([64, 1], F32)
            nc.sync.dma_start(b12_t[:], b12_d[:])
            b21_t = vw.tile([128, 1], F32)
            nc.sync.dma_start(b21_t[:], b21_d[:])
            b22_t = vw.tile([128, 1], F32)
            nc.sync.dma_start(b22_t[:], b22_d[:])
            b31_t = vw.tile([128, 2], F32)
            nc.sync.dma_start(b31_t[:], b31_d[:])
            vmask_t = vw.tile([128, JOBS, 132], F32)
            nc.sync.dma_start(vmask_t[:], vmask_d.rearrange("j p s -> p j s"))

            for jj in range(JOBS):
                # ---- buffers (flat [128, rows*width]) ----
                c11 = vbuf.tile([128, C11R * W322], F32, tag="c11")
                p1 = vbuf.tile([128, P1R * P1W], F32, tag="p1")
                c21 = vbuf.tile([128, C21R * P1W], F32, tag="c21")
                c22 = vbuf.tile([128, C22R * P1W], F32, tag="c22")
                p2 = vbuf.tile([128, P2R * P2W], F32, tag="p2")
                c31 = vbuf.tile([128, 2, 10 * 80], F32, tag="c31")
                nc.gpsimd.memset(c11[:], 0.0)
                nc.gpsimd.memset(p1[:], 0.0)
                nc.gpsimd.memset(c21[:], 0.0)
                nc.gpsimd.memset(p2[:], 0.0)

                # ---- conv1_1: im2col K=27; writes c11 rows [1, 59) ----
                nrows = 58
                r = 0
                while r < nrows:
                    cr = min(CR11, nrows - r)   # chunk c11 rows [1+r, 1+r+cr)
                    imc = vchunk.tile([27, CR11 * W322], F32, tag="vsA")
                    for ki in range(3):
                        for kj in range(3):
                            p0 = (ki * 3 + kj) * 3
                            dst = imc[p0:p0 + 3, :cr * W322].rearrange(
                                "p (r x) -> p r x", x=W322)
                            nc.sync.dma_start(
                                dst, imgp_d[jj, :, r + ki:r + ki + cr, kj:kj + W322])
                    nflat = cr * W322
                    for t0, tw in _subtiles(nflat):
                        ps = vpsum.tile([64, NTW], F32, tag="vps")
                        nc.tensor.matmul(ps[:, :tw], w1c_t[:], imc[:, t0:t0 + tw],
                                         start=True, stop=True)
                        nc.scalar.activation(
                            c11[0:64, (1 + r) * W322 + t0:(1 + r) * W322 + t0 + tw],
                            ps[:, :tw], AF.Relu, bias=b1_t[:])
                    r += cr
                # zero the wrap border pairs (x=321 of row r / x=0 of row r+1)
                c11v = c11[0:64].rearrange("p (r x) -> p r x", x=W322)
                nc.vector.memset(c11v[:, 0:C11R - 1, W322 - 1:W322], 0.0)
                nc.vector.memset(c11v[:, 1:C11R, 0:1], 0.0)
                # zero out-of-image halo rows (image boundary strips only)
                nc.vector.tensor_mul(
                    c11v[:], c11v[:],
                    vmask_t[0:64, jj, 0:C11R].unsqueeze(2).broadcast_to((64, C11R, W322)))
                # r1 out: rows 10..50, cols 1..321
                nc.sync.dma_start(r1_d[jj], c11v[:, 10:50, 1:321])
                # pairing: partitions 64:128 = col+1
                nc.vector.tensor_copy(c11[64:128, 0:C11R * W322 - 1],
                                      c11[0:64, 1:C11R * W322])
                nc.vector.memset(c11[64:128, C11R * W322 - 1:C11R * W322], 0.0)

                # ---- conv1_2 (+pool1): output rows gr in [0,56) ----
                gr = 0
                pu = 1  # pool1 write row
                while gr < 56:
                    cr = min(CR12, 56 - gr)
                    cc = vchunk.tile([64, CR12 * W322], F32, tag="vsA")
                    nflat = cr * W322
                    for t0, tw in _subtiles(nflat):
                        ps = vpsum.tile([64, NTW], F32, tag="vps")
                        for ki in range(3):
                            base = (gr + 1 + ki) * W322 - 1 + t0
                            nc.tensor.matmul(ps[:, :tw], w12p_t[:, ki, :],
                                             c11[:, base:base + tw],
                                             start=(ki == 0), stop=False)
                            nc.tensor.matmul(ps[:, :tw], w12l_t[:, ki, :],
                                             c11[0:64, base + 2:base + 2 + tw],
                                             start=False, stop=(ki == 2))
                        nc.scalar.activation(cc[:, t0:t0 + tw], ps[:, :tw],
                                             AF.Relu, bias=b12_t[:])
                    # pool 2x2 -> p1 rows [pu, pu+cr/2), cols [1,161)
                    ccv2 = cc[:, :nflat].rearrange("p (r two x) -> p r two x",
                                                   two=2, x=W322)
                    rm = vchunk.tile([64, (CR12 // 2) * W322], F32, tag="vsB")
                    rmv = rm[:, :(cr // 2) * W322].rearrange("p (r x) -> p r x", x=W322)
                    nc.vector.tensor_max(rmv[:], ccv2[:, :, 0, :], ccv2[:, :, 1, :])
                    rmp = rmv[:, :, 1:321].rearrange("p r (xh two) -> p r xh two", two=2)
                    p1v = p1[0:64].rearrange("p (r x) -> p r x", x=P1W)
                    nc.vector.tensor_max(p1v[:, pu:pu + cr // 2, 1:161],
                                         rmp[:, :, :, 0], rmp[:, :, :, 1])
                    gr += cr
                    pu += cr // 2
                # mask out-of-image pool1 rows, then pairing
                p1mv = p1[0:64].rearrange("p (r x) -> p r x", x=P1W)
                nc.vector.tensor_mul(
                    p1mv[:], p1mv[:],
                    vmask_t[0:64, jj, 60:60 + P1R].unsqueeze(2).broadcast_to((64, P1R, P1W)))
                nc.vector.tensor_copy(p1[64:128, 0:P1R * P1W - 1],
                                      p1[0:64, 1:P1R * P1W])
                nc.vector.memset(p1[64:128, P1R * P1W - 1:P1R * P1W], 0.0)

                # ---- conv2_1: output c21 rows [1,27), flat n over 26*162 ----
                nflat = 26 * P1W
                for t0, tw in _subtiles(nflat):
                    ps = vpsum.tile([128, NTW], F32, tag="vps")
                    for ki in range(3):
                        base = (1 + ki) * P1W - 1 + t0
                        nc.tensor.matmul(ps[:, :tw], w21p_t[:, ki, :],
                                         p1[:, base:base + tw],
                                         start=(ki == 0), stop=False)
                        nc.tensor.matmul(ps[:, :tw], w21l_t[:, ki, :],
                                         p1[0:64, base + 2:base + 2 + tw],
                                         start=False, stop=(ki == 2))
                    nc.scalar.activation(c21[:, P1W + t0:P1W + t0 + tw], ps[:, :tw],
                                         AF.Relu, bias=b21_t[:])
                c21v = c21[:].rearrange("p (r x) -> p r x", x=P1W)
                nc.vector.memset(c21v[:, 0:C21R - 1, P1W - 1:P1W], 0.0)
                nc.vector.memset(c21v[:, 1:C21R, 0:1], 0.0)
                nc.vector.tensor_mul(
                    c21v[:], c21v[:],
                    vmask_t[:, jj, 90:90 + C21R].unsqueeze(2).broadcast_to((128, C21R, P1W)))
                # r2 out: c21 rows 4..24, cols 1..161
                nc.sync.dma_start(r2_d[jj], c21v[:, 4:24, 1:161])

                # ---- conv2_2: output c22 rows [0,24) == valid ----
                nflat = C22R * P1W
                for t0, tw in _subtiles(nflat):
                    ps = vpsum.tile([128, NTW], F32, tag="vps")
                    for kt in range(9):
                        ki, kj = kt // 3, kt % 3
                        base = (1 + ki) * P1W - 1 + kj + t0
                        nc.tensor.matmul(ps[:, :tw], w22_t[:, kt, :],
                                         c21[:, base:base + tw],
                                         start=(kt == 0), stop=(kt == 8))
                    nc.scalar.activation(c22[:, t0:t0 + tw], ps[:, :tw],
                                         AF.Relu, bias=b22_t[:])
                # pool2 -> p2 rows [1,13), cols [1,81)
                c22v2 = c22[:].rearrange("p (r two x) -> p r two x", two=2, x=P1W)
                rm2 = vchunk.tile([128, 12 * P1W], F32, tag="vsB")
                rm2v = rm2[:].rearrange("p (r x) -> p r x", x=P1W)
                nc.vector.tensor_max(rm2v[:], c22v2[:, :, 0, :], c22v2[:, :, 1, :])
                rm2p = rm2v[:, :, 1:161].rearrange("p r (xh two) -> p r xh two", two=2)
                p2v = p2[:].rearrange("p (r x) -> p r x", x=P2W)
                nc.vector.tensor_max(p2v[:, 1:13, 1:81],
                                     rm2p[:, :, :, 0], rm2p[:, :, :, 1])
                nc.vector.tensor_mul(
                    p2v[:], p2v[:],
                    vmask_t[:, jj, 118:118 + P2R].unsqueeze(2).broadcast_to((128, P2R, P2W)))

                # ---- conv3_1: out rows [0,10), N = 10*82 in 2 tiles of 5 rows ----
                for half in range(2):
                    for u0 in (0, 5):
                        ps = vpsum.tile([128, 5 * P2W], F32, tag="vps")
                        for kt in range(9):
                            ki, kj = kt // 3, kt % 3
                            base = (u0 + 1 + ki) * P2W - 1 + kj
                            nc.tensor.matmul(ps[:], w31_t[:, 2 * kt + half, :],
                                             p2[:, base:base + 5 * P2W],
                                             start=(kt == 0), stop=(kt == 8))
                        psv = ps[:].rearrange("p (r x) -> p r x", x=P2W)
                        outv = c31[:, half].rearrange("p (r x) -> p r x", x=80)
                        nc.scalar.activation(outv[:, u0:u0 + 5, :],
                                             psv[:, :, 1:81], AF.Relu,
                                             bias=b31_t[:, half:half + 1])
                    nc.sync.dma_start(
                        r3_d[jj, half],
                        c31[:, half].rearrange("p (r x) -> p r x", x=80))

    return nc


# ---------------- host-side helpers ----------------
def _prep_inputs(dense_features1, dense_features2, img_ref_hr, vgg_params):
    d1 = np.ascontiguousarray(dense_features1, np.float32).reshape(B, C, PIX)
    d2 = np.ascontiguousarray(dense_features2, np.float32).reshape(B, C, PIX)
    img = np.ascontiguousarray(img_ref_hr, np.float32)

    mean = np.array([0.485, 0.456, 0.406], np.float32).reshape(3, 1, 1)
    std = np.array([0.229, 0.224, 0.225], np.float32).reshape(3, 1, 1)
    w11 = np.asarray(vgg_params["w1_1"], np.float32)
    b1f = np.asarray(vgg_params["b1_1"], np.float32)
    # image is normalized on host (padding must be zero in normalized domain)
    img = (img - mean[None]) / std[None]
    # layout [ (ki*3+kj)*3 + c, o ]
    w1c = np.ascontiguousarray(
        w11.transpose(2, 3, 1, 0).reshape(27, 64))

    def pair_weights(wkey, cin):
        wmat = np.asarray(vgg_params[wkey], np.float32)  # [o, i, 3, 3]
        cout = wmat.shape[0]
        wp = np.zeros((128, 3, cout), np.float32)
        wl = np.zeros((64, 3, cout), np.float32)
        for ki in range(3):
            wp[:cin, ki] = wmat[:, :, ki, 0].T
            wp[64:64 + cin, ki] = wmat[:, :, ki, 1].T
            wl[:cin, ki] = wmat[:, :, ki, 2].T
        return np.ascontiguousarray(wp), np.ascontiguousarray(wl)

    w12p, w12l = pair_weights("w1_2", 64)
    w21p, w21l = pair_weights("w2_1", 64)
    w22m = np.asarray(vgg_params["w2_2"], np.float32)
    w22 = np.ascontiguousarray(
        w22m.transpose(2, 3, 1, 0).reshape(9, 128, 128).transpose(1, 0, 2))
    w31m = np.asarray(vgg_params["w3_1"], np.float32)  # [256,128,3,3]
    w31 = np.zeros((128, 18, 128), np.float32)
    for kt in range(9):
        ki, kj = kt // 3, kt % 3
        w31[:, 2 * kt + 0] = w31m[0:128, :, ki, kj].T
        w31[:, 2 * kt + 1] = w31m[128:256, :, ki, kj].T

    common = {
        "w1c": w1c, "b1": b1f.reshape(64, 1).astype(np.float32),
        "w12p": w12p, "w12l": w12l,
        "b12": np.asarray(vgg_params["b1_2"], np.float32).reshape(64, 1),
        "w21p": w21p, "w21l": w21l,
        "b21": np.asarray(vgg_params["b2_1"], np.float32).reshape(128, 1),
        "w22": w22,
        "b22": np.asarray(vgg_params["b2_2"], np.float32).reshape(128, 1),
        "w31": np.ascontiguousarray(w31),
        "b31": np.ascontiguousarray(
            np.asarray(vgg_params["b3_1"], np.float32).reshape(2, 128).T),
    }

    in_maps = []
    for c in range(8):
        b, q = c // 4, c % 4
        fin = np.zeros((C, FINW), np.float32)
        lo = q * MPC
        hi = min(PIX, lo + FINW)
        fin[:, :hi - lo] = d1[b, :, lo:hi]
        fref = np.zeros((C, FREFW), np.float32)
        fref[:, :PIX] = d2[b]
        imgp = np.zeros((JOBS, 3, C11R, W328), np.float32)
        vmask = np.zeros((JOBS, 128, 132), np.float32)
        for j in range(JOBS):
            s = 2 * q + j
            g0 = 40 * s - 10       # imgp row 0 <-> image row g0
            r0 = max(0, -g0)
            r1 = min(C11R, 320 - g0)
            imgp[j, :, r0:r1, 2:322] = img[b, :, g0 + r0:g0 + r1, :]
            segs = [(0, C11R, 40 * s - 10, 320),    # c11: row lr <-> 40s-10+lr
                    (60, P1R, 20 * s - 5, 160),     # p1
                    (90, C21R, 20 * s - 4, 160),    # c21
                    (118, P2R, 10 * s - 2, 80)]     # p2
            for o0, n, gg0, lim in segs:
                rows = gg0 + np.arange(n)
                vmask[j, :, o0:o0 + n] = ((rows >= 0) & (rows < lim)).astype(np.float32)
        m = dict(common)
        m["fin"] = fin
        m["fref"] = fref
        m["imgp"] = imgp
        m["vmask"] = vmask
        in_maps.append(m)
    return in_maps


def _shift_np(x, si, sj):
    # shift content down/right with zero fill; x: [B,K,H,W,2]
    return np.pad(x, ((0, 0), (0, 0), (si, 0), (sj, 0), (0, 0)))[:, :, :x.shape[2], :x.shape[3], :]


def _postprocess(results):
    K = 3
    top_idx = np.zeros((B, K, W2, W2), np.int64)
    top_val = np.zeros((B, K, W2, W2), np.float32)
    m = np.arange(NREF)
    valid = (m % W) < W2
    scale = np.float32(1.0) / np.float32((3.0 + EPS_PATCH) * (3.0 + EPS_PATCH))
    for b in range(B):
        tv = np.concatenate([results[4 * b + q]["topv"] for q in range(4)], 0)[:NREF]
        ti = np.concatenate([results[4 * b + q]["topi"] for q in range(4)], 0)[:NREF]
        g = ti[valid][:, :K].astype(np.int64)
        idx78 = (g // W) * W2 + (g % W)
        v = tv[valid][:, :K].astype(np.float32) * scale
        top_idx[b] = idx78.reshape(W2, W2, K).transpose(2, 0, 1)
        top_val[b] = v.reshape(W2, W2, K).transpose(2, 0, 1)

    fw = (top_idx % W2).astype(np.float32)
    fh = (top_idx // W2).astype(np.float32)
    gx = np.arange(W2, dtype=np.float32)
    gy = np.arange(W2, dtype=np.float32)
    flow = np.stack([fw - gx[None, None, None, :], fh - gy[None, None, :, None]], -1)
    flow3 = np.pad(flow, ((0, 0), (0, 0), (0, 2), (0, 2), (0, 0)))
    off3 = np.stack([_shift_np(flow3, i, j) for i in range(3) for j in range(3)], 2)
    sim3 = np.pad(top_val, ((0, 0), (0, 0), (1, 1), (1, 1)))

    flow2 = np.repeat(np.repeat(flow3, 2, 2), 2, 3) * np.float32(2.0)
    off2 = np.stack([_shift_np(flow2, 2 * i, 2 * j) for i in range(3) for j in range(3)], 2)
    sim2 = np.repeat(np.repeat(sim3, 2, 2), 2, 3)

    flow1 = np.repeat(np.repeat(flow3, 4, 2), 4, 3) * np.float32(4.0)
    off1 = np.stack([_shift_np(flow1, 4 * i, 4 * j) for i in range(3) for j in range(3)], 2)
    sim1 = np.repeat(np.repeat(sim3, 4, 2), 4, 3)

    r1 = np.zeros((B, 64, 320, 320), np.float32)
    r2 = np.zeros((B, 128, 160, 160), np.float32)
    r3 = np.zeros((B, 256, 80, 80), np.float32)
    for c in range(8):
        b, q = c // 4, c % 4
        for j in range(JOBS):
            s = 2 * q + j
            r1[b, :, 40 * s:40 * s + 40, :] = results[c]["r1o"][j]
            r2[b, :, 20 * s:20 * s + 20, :] = results[c]["r2o"][j]
            r3[b, 0:128, 10 * s:10 * s + 10, :] = results[c]["r3o"][j, 0]
            r3[b, 128:256, 10 * s:10 * s + 10, :] = results[c]["r3o"][j, 1]

    return (flow1.astype(np.float32), flow2.astype(np.float32), flow3.astype(np.float32),
            off1.astype(np.float32), off2.astype(np.float32), off3.astype(np.float32),
            sim1, sim2, sim3, r1, r2, r3)


LAST_RUN_SECONDS = None


def kernel(dense_features1, dense_features2, img_ref_hr, vgg_params):
    import time
    global LAST_RUN_SECONDS
    if "nc" not in _CACHE:
        nc = _build_program()
        _split_excess_waits(nc)  # hardware codegen: <=1 sync wait per inst
        _CACHE["nc"] = nc
    nc = _CACHE["nc"]
    in_maps = _prep_inputs(dense_features1, dense_features2, img_ref_hr, vgg_params)
    t0 = time.time()
    res = run_bass_kernel_spmd(nc, in_maps, list(range(8)))
    LAST_RUN_SECONDS = time.time() - t0
    return _postprocess(res.results)


# revision 21
# speedup vs baseline: 1.1854x; 1.1854x over previous
"""Trainium2 Bass kernel for CorrespondenceGenerationArch.

Per-core (8 cores = 2 samples x 4 quarters):
  - per-pixel L2 feature normalization (on device)
  - correlation GEMM [1664 x 2304] @ [2304 x 6240] (fp32, PE)
  - top-8 values+indices per output row (DVE max/max_index)
  - VGG head (conv1_1..conv3_1) on 2 spatial strips
Host: input slicing/padding, weight folding, flow/offset/sim expansion from
the top-3 indices/values (pure index arithmetic), output assembly.
"""
import sys

if "/opt/trn_rl_repo" not in sys.path:
    sys.path.insert(0, "/opt/trn_rl_repo")

import numpy as np

import concourse.bass as bass
import concourse.mybir as mybir
import concourse.tile as tile
from concourse.bass_utils import run_bass_kernel_spmd

F32 = mybir.dt.float32
U32 = mybir.dt.uint32
AF = mybir.ActivationFunctionType

# ---------------- problem constants (hardcoded) ----------------
B, C, H, W = 2, 256, 80, 80
PIX = H * W                      # 6400
W2 = H - 2                       # 78
NREF = W2 * W2                   # 6084 valid ref patches
NT, NTW = 13, 480                # N tiles (GEMM uses NTE=468 = 6 rows x 78)
NTE = 468
MC = 13                          # M chunks of 128 per core
MPC = MC * 128                   # 1664 M rows per core
FINW = 1856                      # fin slice width (1664 + 162 pad -> 1856)
FREFW = 6592                     # fref width (6400 + 162 pad -> 6592)
DKS = [di * W + dj for di in range(3) for dj in range(3)]  # patch offsets
EPS_PATCH = 1e-5

# VGG strip geometry (per job; 16 strips of 40 rows at 320-res, 2 jobs/core)
JOBS = 2
W322, W328 = 322, 328
C11R = 60          # c11 buffer rows (58 valid + slack)
CR11 = 10          # conv1_1 chunk rows
CR12 = 12          # conv1_2 chunk rows (must be even)
P1R, P1W = 30, 162   # pool1 buffer rows/width (28 valid + 2)
C21R = 28          # conv2_1 buffer rows (26 valid + 2)
C22R = 24          # conv2_2 rows (exactly valid)
P2R, P2W = 14, 82    # pool2 rows/width (12 valid + 2)

_CACHE = {}


# ---------------- BIR post-pass: walrus accepts 1 sync-wait/instruction ----
def _split_excess_waits(nc, cap=1):
    n = 0
    for f in nc.m.functions:
        for bb in f.blocks:
            il = bb.instructions
            out = []
            changed = False
            for ins in il:
                si = getattr(ins, "sync_info", None)
                ow = list(si.on_wait) if si is not None and si.on_wait else []
                k = 0
                while len(ow) > cap:
                    chunk, ow = ow[:cap], ow[cap:]
                    out.append(mybir.InstNoOp(
                        name=f"{ins.name}_ws{k}",
                        sync_info=mybir.SyncInfo(on_wait=chunk, on_update=[]),
                        engine=ins.engine,
                        bass_nofuse=True,
                    ))
                    k += 1
                    n += 1
                if k:
                    si.on_wait = ow
                    changed = True
                out.append(ins)
            if changed:
                il[:] = out
    return n


# ---------------- device program ----------------
def _normalize(nc, tc, pool, psum, f_t, width, subtiles, ones_col, ones_row):
    """Per-pixel (column) L2-normalize f_t[2][128, width] in place."""
    row_pool = pool
    sums = row_pool.tile([1, width], F32, tag="normrow")
    for s0, sw in subtiles:
        sq = row_pool.tile([128, NTW], F32, tag="sqtmp")
        ps = psum.tile([1, NTW], F32, tag="normps")
        for ch in range(2):
            nc.vector.tensor_mul(sq[:, :sw], f_t[ch][:, s0:s0 + sw], f_t[ch][:, s0:s0 + sw])
            nc.tensor.matmul(ps[:, :sw], ones_col[:], sq[:, :sw],
                             start=(ch == 0), stop=(ch == 1))
        nc.vector.tensor_copy(sums[:, s0:s0 + sw], ps[:, :sw])
    # norm = max(sqrt(sumsq), 1e-12); r = 1/norm
    nc.scalar.activation(sums[:], sums[:], AF.Sqrt)
    nc.vector.tensor_scalar_max(sums[:], sums[:], 1.0e-12)
    nc.vector.reciprocal(sums[:], sums[:])
    for s0, sw in subtiles:
        bc = psum.tile([128, NTW], F32, tag="bcps")
        nc.tensor.matmul(bc[:, :sw], ones_row[:], sums[:, s0:s0 + sw],
                         start=True, stop=True)
        for ch in range(2):
            nc.vector.tensor_mul(f_t[ch][:, s0:s0 + sw], f_t[ch][:, s0:s0 + sw], bc[:, :sw])


def _subtiles(width):
    out = []
    s = 0
    while s < width:
        out.append((s, min(NTW, width - s)))
        s += NTW
    return out


def _build_program():
    nc = bass.Bass()

    fin_d = nc.dram_tensor("fin", [C, FINW], F32, kind="ExternalInput")
    fref_d = nc.dram_tensor("fref", [C, FREFW], F32, kind="ExternalInput")
    imgp_d = nc.dram_tensor("imgp", [JOBS, 3, C11R, W328], F32, kind="ExternalInput")
    w1c_d = nc.dram_tensor("w1c", [27, 64], F32, kind="ExternalInput")
    b1_d = nc.dram_tensor("b1", [64, 1], F32, kind="ExternalInput")
    w12p_d = nc.dram_tensor("w12p", [128, 3, 64], F32, kind="ExternalInput")
    w12l_d = nc.dram_tensor("w12l", [64, 3, 64], F32, kind="ExternalInput")
    b12_d = nc.dram_tensor("b12", [64, 1], F32, kind="ExternalInput")
    w21p_d = nc.dram_tensor("w21p", [128, 3, 128], F32, kind="ExternalInput")
    w21l_d = nc.dram_tensor("w21l", [64, 3, 128], F32, kind="ExternalInput")
    b21_d = nc.dram_tensor("b21", [128, 1], F32, kind="ExternalInput")
    w22_d = nc.dram_tensor("w22", [128, 9, 128], F32, kind="ExternalInput")
    b22_d = nc.dram_tensor("b22", [128, 1], F32, kind="ExternalInput")
    w31_d = nc.dram_tensor("w31", [128, 18, 128], F32, kind="ExternalInput")
    b31_d = nc.dram_tensor("b31", [128, 2], F32, kind="ExternalInput")
    # per-level out-of-image row masks: segments c11[0:60] p1[60:90] c21[90:118] p2[118:132]
    vmask_d = nc.dram_tensor("vmask", [JOBS, 128, 132], F32, kind="ExternalInput")

    topv_d = nc.dram_tensor("topv", [MPC, 8], F32, kind="ExternalOutput")
    topi_d = nc.dram_tensor("topi", [MPC, 8], U32, kind="ExternalOutput")
    r1_d = nc.dram_tensor("r1o", [JOBS, 64, 40, 320], F32, kind="ExternalOutput")
    r2_d = nc.dram_tensor("r2o", [JOBS, 128, 20, 160], F32, kind="ExternalOutput")
    r3_d = nc.dram_tensor("r3o", [JOBS, 2, 128, 10, 80], F32, kind="ExternalOutput")

    with tile.TileContext(nc) as tc:
        # ---------------- phase 1: correlation + topk ----------------
        F16 = mybir.dt.float16
        with tc.tile_pool(name="feat", bufs=1) as feat, \
             tc.tile_pool(name="small", bufs=2) as small, \
             tc.tile_pool(name="gpsum", bufs=2, space="PSUM") as gpsum:

            # fp16 hi/lo split of the normalized features (hi+lo == fp32 value
            # to ~2^-22): 3 fp16 matmul passes run at 1 cyc/row vs fp32's 4.
            fin_h = [feat.tile([128, FINW], F16, tag=f"finh{ch}", name=f"finh{ch}") for ch in range(2)]
            fin_l = [feat.tile([128, FINW], F16, tag=f"finl{ch}", name=f"finl{ch}") for ch in range(2)]
            fref_h = [feat.tile([128, FREFW], F16, tag=f"frefh{ch}", name=f"frefh{ch}") for ch in range(2)]
            fref_l = [feat.tile([128, FREFW], F16, tag=f"frefl{ch}", name=f"frefl{ch}") for ch in range(2)]

            with tc.tile_pool(name="rawf", bufs=1) as rawf:
                ones_col = rawf.tile([128, 1], F32)
                nc.vector.memset(ones_col[:], 1.0)
                ones_row = rawf.tile([1, 128], F32)
                nc.vector.memset(ones_row[:], 1.0)
                fin_t = [rawf.tile([128, FINW], F32, tag=f"fin{ch}", name=f"fin{ch}") for ch in range(2)]
                fref_t = [rawf.tile([128, FREFW], F32, tag=f"fref{ch}", name=f"fref{ch}") for ch in range(2)]
                for ch in range(2):
                    for s0, sw in _subtiles(FINW):
                        nc.sync.dma_start(fin_t[ch][:, s0:s0 + sw],
                                          fin_d[ch * 128:(ch + 1) * 128, s0:s0 + sw])
                    for s0, sw in _subtiles(FREFW):
                        nc.sync.dma_start(fref_t[ch][:, s0:s0 + sw],
                                          fref_d[ch * 128:(ch + 1) * 128, s0:s0 + sw])

                _normalize(nc, tc, small, gpsum, fin_t, FINW, _subtiles(FINW), ones_col, ones_row)
                _normalize(nc, tc, small, gpsum, fref_t, FREFW, _subtiles(FREFW), ones_col, ones_row)

                for ch in range(2):
                    for f32t, h, lo, width in ((fin_t[ch], fin_h[ch], fin_l[ch], FINW),
                                               (fref_t[ch], fref_h[ch], fref_l[ch], FREFW)):
                        for s0, sw in _subtiles(width):
                            tmp = small.tile([128, NTW], F32, tag="sqtmp", name="spl")
                            nc.vector.tensor_copy(h[:, s0:s0 + sw], f32t[:, s0:s0 + sw])
                            nc.vector.tensor_copy(tmp[:, :sw], h[:, s0:s0 + sw])
                            nc.vector.tensor_sub(tmp[:, :sw], f32t[:, s0:s0 + sw], tmp[:, :sw])
                            nc.vector.tensor_copy(lo[:, s0:s0 + sw], tmp[:, :sw])

            corr_cm = tc.tile_pool(name="corr", bufs=2)
            corrp = corr_cm.__enter__()
            for mc in range(MC):
                m0 = mc * 128
                corr = corrp.tile([128, NREF], F32, tag="corr")
                for j in range(NT):
                    n0pix = j * 6 * W
                    pt = gpsum.tile([128, NTE], F32, tag="gemm")
                    n = 0
                    for fa, fb in ((fin_h, fref_h), (fin_h, fref_l), (fin_l, fref_h)):
                        for kt in range(18):
                            ch, dk = kt // 9, DKS[kt % 9]
                            n += 1
                            rhs = fb[ch][:, n0pix + dk:n0pix + dk + 6 * W] \
                                .rearrange("p (r c) -> p r c", c=W)[:, :, 0:W2]
                            nc.tensor.matmul(
                                pt[:],
                                fa[ch][:, m0 + dk:m0 + dk + 128],
                                rhs,
                                start=(n == 1), stop=(n == 54))
                    nc.vector.tensor_copy(corr[:, j * NTE:(j + 1) * NTE], pt[:])
                tv = small.tile([128, 8], F32, tag="tv")
                ti = small.tile([128, 8], U32, tag="ti")
                nc.vector.max(out=tv[:], in_=corr[:])
                nc.vector.max_index(out=ti[:], in_max=tv[:], in_values=corr[:])
                nc.sync.dma_start(topv_d[m0:m0 + 128, :], tv[:])
                nc.sync.dma_start(topi_d[m0:m0 + 128, :], ti[:])
            corr_cm.__exit__(None, None, None)

        # ---------------- phase 2: VGG head ----------------
        with tc.tile_pool(name="vw", bufs=1) as vw, \
             tc.tile_pool(name="vbuf", bufs=1) as vbuf, \
             tc.tile_pool(name="vchunk", bufs=2) as vchunk, \
             tc.tile_pool(name="vpsum", bufs=4, space="PSUM") as vpsum:

            w1c_t = vw.tile([27, 64], F32)
            nc.sync.dma_start(w1c_t[:], w1c_d[:])
            w12p_t = vw.tile([128, 3, 64], F32)
            nc.sync.dma_start(w12p_t[:], w12p_d[:])
            w12l_t = vw.tile([64, 3, 64], F32)
            nc.sync.dma_start(w12l_t[:], w12l_d[:])
            w21p_t = vw.tile([128, 3, 128], F32)
            nc.sync.dma_start(w21p_t[:], w21p_d[:])
            w21l_t = vw.tile([64, 3, 128], F32)
            nc.sync.dma_start(w21l_t[:], w21l_d[:])
            w22_t = vw.tile([128, 9, 128], F32)
            nc.sync.dma_start(w22_t[:], w22_d[:])
            w31_t = vw.tile([128, 18, 128], F32)
            nc.sync.dma_start(w31_t[:], w31_d[:])
            b1_t = vw.tile([64, 1], F32)
            nc.sync.dma_start(b1_t[:], b1_d[:])
            b12_t = vw.tile([64, 1], F32)
            nc.sync.dma_start(b12_t[:], b12_d[:])
            b21_t = vw.tile([128, 1], F32)
            nc.sync.dma_start(b21_t[:], b21_d[:])
            b22_t = vw.tile([128, 1], F32)
            nc.sync.dma_start(b22_t[:], b22_d[:])
            b31_t = vw.tile([128, 2], F32)
            nc.sync.dma_start(b31_t[:], b31_d[:])
            vmask_t = vw.tile([128, JOBS, 132], F32)
            nc.sync.dma_start(vmask_t[:], vmask_d.rearrange("j p s -> p j s"))

            for jj in range(JOBS):
                # ---- buffers (flat [128, rows*width]) ----
                c11 = vbuf.tile([128, C11R * W322], F32, tag="c11")
                p1 = vbuf.tile([128, P1R * P1W], F32, tag="p1")
                c21 = vbuf.tile([128, C21R * P1W], F32, tag="c21")
                c22 = vbuf.tile([128, C22R * P1W], F32, tag="c22")
                p2 = vbuf.tile([128, P2R * P2W], F32, tag="p2")
                c31 = vbuf.tile([128, 2, 10 * 80], F32, tag="c31")
                nc.gpsimd.memset(c11[:], 0.0)
                nc.gpsimd.memset(p1[:], 0.0)
                nc.gpsimd.memset(c21[:], 0.0)
                nc.gpsimd.memset(p2[:], 0.0)

                # ---- conv1_1: im2col K=27; writes c11 rows [1, 59) ----
                nrows = 58
                r = 0
                while r < nrows:
                    cr = min(CR11, nrows - r)   # chunk c11 rows [1+r, 1+r+cr)
                    imc = vchunk.tile([27, CR11 * W322], F32, tag="vsA")
                    for ki in range(3):
                        for kj in range(3):
                            p0 = (ki * 3 + kj) * 3
                            dst = imc[p0:p0 + 3, :cr * W322].rearrange(
                                "p (r x) -> p r x", x=W322)
                            nc.sync.dma_start(
                                dst, imgp_d[jj, :, r + ki:r + ki + cr, kj:kj + W322])
                    nflat = cr * W322
                    for t0, tw in _subtiles(nflat):
                        ps = vpsum.tile([64, NTW], F32, tag="vps")
                        nc.tensor.matmul(ps[:, :tw], w1c_t[:], imc[:, t0:t0 + tw],
                                         start=True, stop=True)
                        nc.scalar.activation(
                            c11[0:64, (1 + r) * W322 + t0:(1 + r) * W322 + t0 + tw],
                            ps[:, :tw], AF.Relu, bias=b1_t[:])
                    r += cr
                # zero the wrap border pairs (x=321 of row r / x=0 of row r+1)
                c11v = c11[0:64].rearrange("p (r x) -> p r x", x=W322)
                nc.vector.memset(c11v[:, 0:C11R - 1, W322 - 1:W322], 0.0)
                nc.vector.memset(c11v[:, 1:C11R, 0:1], 0.0)
                # zero out-of-image halo rows (image boundary strips only)
                nc.vector.tensor_mul(
                    c11v[:], c11v[:],
                    vmask_t[0:64, jj, 0:C11R].unsqueeze(2).broadcast_to((64, C11R, W322)))
                # r1 out: rows 10..50, cols 1..321
                nc.sync.dma_start(r1_d[jj], c11v[:, 10:50, 1:321])
                # pairing: partitions 64:128 = col+1
                nc.vector.tensor_copy(c11[64:128, 0:C11R * W322 - 1],
                                      c11[0:64, 1:C11R * W322])
                nc.vector.memset(c11[64:128, C11R * W322 - 1:C11R * W322], 0.0)

                # ---- conv1_2 (+pool1): output rows gr in [0,56) ----
                gr = 0
                pu = 1  # pool1 write row
                while gr < 56:
                    cr = min(CR12, 56 - gr)
                    cc = vchunk.tile([64, CR12 * W322], F32, tag="vsA")
                    nflat = cr * W322
                    for t0, tw in _subtiles(nflat):
                        ps = vpsum.tile([64, NTW], F32, tag="vps")
                        for ki in range(3):
                            base = (gr + 1 + ki) * W322 - 1 + t0
                            nc.tensor.matmul(ps[:, :tw], w12p_t[:, ki, :],
                                             c11[:, base:base + tw],
                                             start=(ki == 0), stop=False)
                            nc.tensor.matmul(ps[:, :tw], w12l_t[:, ki, :],
                                             c11[0:64, base + 2:base + 2 + tw],
                                             start=False, stop=(ki == 2))
                        nc.scalar.activation(cc[:, t0:t0 + tw], ps[:, :tw],
                                             AF.Relu, bias=b12_t[:])
                    # pool 2x2 -> p1 rows [pu, pu+cr/2), cols [1,161)
                    ccv2 = cc[:, :nflat].rearrange("p (r two x) -> p r two x",
                                                   two=2, x=W322)
                    rm = vchunk.tile([64, (CR12 // 2) * W322], F32, tag="vsB")
                    rmv = rm[:, :(cr // 2) * W322].rearrange("p (r x) -> p r x", x=W322)
                    nc.vector.tensor_max(rmv[:], ccv2[:, :, 0, :], ccv2[:, :, 1, :])
                    rmp = rmv[:, :, 1:321].rearrange("p r (xh two) -> p r xh two", two=2)
                    p1v = p1[0:64].rearrange("p (r x) -> p r x", x=P1W)
                    nc.vector.tensor_max(p1v[:, pu:pu + cr // 2, 1:161],
                                         rmp[:, :, :, 0], rmp[:, :, :, 1])
                    gr += cr
                    pu += cr // 2
                # mask out-of-image pool1 rows, then pairing
                p1mv = p1[0:64].rearrange("p (r x) -> p r x", x=P1W)
                nc.vector.tensor_mul(
                    p1mv[:], p1mv[:],
                    vmask_t[0:64, jj, 60:60 + P1R].unsqueeze(2).broadcast_to((64, P1R, P1W)))
                nc.vector.tensor_copy(p1[64:128, 0:P1R * P1W - 1],
                                      p1[0:64, 1:P1R * P1W])
                nc.vector.memset(p1[64:128, P1R * P1W - 1:P1R * P1W], 0.0)

                # ---- conv2_1: output c21 rows [1,27), flat n over 26*162 ----
                nflat = 26 * P1W
                for t0, tw in _subtiles(nflat):
                    ps = vpsum.tile([128, NTW], F32, tag="vps")
                    for ki in range(3):
                        base = (1 + ki) * P1W - 1 + t0
                        nc.tensor.matmul(ps[:, :tw], w21p_t[:, ki, :],
                                         p1[:, base:base + tw],
                                         start=(ki == 0), stop=False)
                        nc.tensor.matmul(ps[:, :tw], w21l_t[:, ki, :],
                                         p1[0:64, base + 2:base + 2 + tw],
                                         start=False, stop=(ki == 2))
                    nc.scalar.activation(c21[:, P1W + t0:P1W + t0 + tw], ps[:, :tw],
                                         AF.Relu, bias=b21_t[:])
                c21v = c21[:].rearrange("p (r x) -> p r x", x=P1W)
                nc.vector.memset(c21v[:, 0:C21R - 1, P1W - 1:P1W], 0.0)
                nc.vector.memset(c21v[:, 1:C21R, 0:1], 0.0)
                nc.vector.tensor_mul(
                    c21v[:], c21v[:],
                    vmask_t[:, jj, 90:90 + C21R].unsqueeze(2).broadcast_to((128, C21R, P1W)))
                # r2 out: c21 rows 4..24, cols 1..161
                nc.sync.dma_start(r2_d[jj], c21v[:, 4:24, 1:161])

                # ---- conv2_2: output c22 rows [0,24) == valid ----
                nflat = C22R * P1W
                for t0, tw in _subtiles(nflat):
                    ps = vpsum.tile([128, NTW], F32, tag="vps")
                    for kt in range(9):
                        ki, kj = kt // 3, kt % 3
                        base = (1 + ki) * P1W - 1 + kj + t0
                        nc.tensor.matmul(ps[:, :tw], w22_t[:, kt, :],
                                         c21[:, base:base + tw],
                                         start=(kt == 0), stop=(kt == 8))
                    nc.scalar.activation(c22[:, t0:t0 + tw], ps[:, :tw],
                                         AF.Relu, bias=b22_t[:])
                # pool2 -> p2 rows [1,13), cols [1,81)
                c22v2 = c22[:].rearrange("p (r two x) -> p r two x", two=2, x=P1W)
                rm2 = vchunk.tile([128, 12 * P1W], F32, tag="vsB")
                rm2v = rm2[:].rearrange("p (r x) -> p r x", x=P1W)
                nc.vector.tensor_max(rm2v[:], c22v2[:, :, 0, :], c22v2[:, :, 1, :])
                rm2p = rm2v[:, :, 1:161].rearrange("p r (xh two) -> p r xh two", two=2)
                p2v = p2[:].rearrange("p (r x) -> p r x", x=P2W)
                nc.vector.tensor_max(p2v[:, 1:13, 1:81],
                                     rm2p[:, :, :, 0], rm2p[:, :, :, 1])
                nc.vector.tensor_mul(
                    p2v[:], p2v[:],
                    vmask_t[:, jj, 118:118 + P2R].unsqueeze(2).broadcast_to((128, P2R, P2W)))

                # ---- conv3_1: out rows [0,10), N = 10*82 in 2 tiles of 5 rows ----
                for half in range(2):
                    for u0 in (0, 5):
                        ps = vpsum.tile([128, 5 * P2W], F32, tag="vps")
                        for kt in range(9):
                            ki, kj = kt // 3, kt % 3
                            base = (u0 + 1 + ki) * P2W - 1 + kj
                            nc.tensor.matmul(ps[:], w31_t[:, 2 * kt + half, :],
                                             p2[:, base:base + 5 * P2W],
                                             start=(kt == 0), stop=(kt == 8))
                        psv = ps[:].rearrange("p (r x) -> p r x", x=P2W)
                        outv = c31[:, half].rearrange("p (r x) -> p r x", x=80)
                        nc.scalar.activation(outv[:, u0:u0 + 5, :],
                                             psv[:, :, 1:81], AF.Relu,
                                             bias=b31_t[:, half:half + 1])
                    nc.sync.dma_start(
                        r3_d[jj, half],
                        c31[:, half].rearrange("p (r x) -> p r x", x=80))

    return nc


# ---------------- host-side helpers ----------------
def _prep_inputs(dense_features1, dense_features2, img_ref_hr, vgg_params):
    d1 = np.ascontiguousarray(dense_features1, np.float32).reshape(B, C, PIX)
    d2 = np.ascontiguousarray(dense_features2, np.float32).reshape(B, C, PIX)
    img = np.ascontiguousarray(img_ref_hr, np.float32)

    mean = np.array([0.485, 0.456, 0.406], np.float32).reshape(3, 1, 1)
    std = np.array([0.229, 0.224, 0.225], np.float32).reshape(3, 1, 1)
    w11 = np.asarray(vgg_params["w1_1"], np.float32)
    b1f = np.asarray(vgg_params["b1_1"], np.float32)
    # image is normalized on host (padding must be zero in normalized domain)
    img = (img - mean[None]) / std[None]
    # layout [ (ki*3+kj)*3 + c, o ]
    w1c = np.ascontiguousarray(
        w11.transpose(2, 3, 1, 0).reshape(27, 64))

    def pair_weights(wkey, cin):
        wmat = np.asarray(vgg_params[wkey], np.float32)  # [o, i, 3, 3]
        cout = wmat.shape[0]
        wp = np.zeros((128, 3, cout), np.float32)
        wl = np.zeros((64, 3, cout), np.float32)
        for ki in range(3):
            wp[:cin, ki] = wmat[:, :, ki, 0].T
            wp[64:64 + cin, ki] = wmat[:, :, ki, 1].T
            wl[:cin, ki] = wmat[:, :, ki, 2].T
        return np.ascontiguousarray(wp), np.ascontiguousarray(wl)

    w12p, w12l = pair_weights("w1_2", 64)
    w21p, w21l = pair_weights("w2_1", 64)
    w22m = np.asarray(vgg_params["w2_2"], np.float32)
    w22 = np.ascontiguousarray(
        w22m.transpose(2, 3, 1, 0).reshape(9, 128, 128).transpose(1, 0, 2))
    w31m = np.asarray(vgg_params["w3_1"], np.float32)  # [256,128,3,3]
    w31 = np.zeros((128, 18, 128), np.float32)
    for kt in range(9):
        ki, kj = kt // 3, kt % 3
        w31[:, 2 * kt + 0] = w31m[0:128, :, ki, kj].T
        w31[:, 2 * kt + 1] = w31m[128:256, :, ki, kj].T

    common = {
        "w1c": w1c, "b1": b1f.reshape(64, 1).astype(np.float32),
        "w12p": w12p, "w12l": w12l,
        "b12": np.asarray(vgg_params["b1_2"], np.float32).reshape(64, 1),
        "w21p": w21p, "w21l": w21l,
        "b21": np.asarray(vgg_params["b2_1"], np.float32).reshape(128, 1),
        "w22": w22,
        "b22": np.asarray(vgg_params["b2_2"], np.float32).reshape(128, 1),
        "w31": np.ascontiguousarray(w31),
        "b31": np.ascontiguousarray(
            np.asarray(vgg_params["b3_1"], np.float32).reshape(2, 128).T),
    }

    in_maps = []
    for c in range(8):
        b, q = c // 4, c % 4
        fin = np.zeros((C, FINW), np.float32)
        lo = q * MPC
        hi = min(PIX, lo + FINW)
        fin[:, :hi - lo] = d1[b, :, lo:hi]
        fref = np.zeros((C, FREFW), np.float32)
        fref[:, :PIX] = d2[b]
        imgp = np.zeros((JOBS, 3, C11R, W328), np.float32)
        vmask = np.zeros((JOBS, 128, 132), np.float32)
        for j in range(JOBS):
            s = 2 * q + j
            g0 = 40 * s - 10       # imgp row 0 <-> image row g0
            r0 = max(0, -g0)
            r1 = min(C11R, 320 - g0)
            imgp[j, :, r0:r1, 2:322] = img[b, :, g0 + r0:g0 + r1, :]
            segs = [(0, C11R, 40 * s - 10, 320),    # c11: row lr <-> 40s-10+lr
                    (60, P1R, 20 * s - 5, 160),     # p1
                    (90, C21R, 20 * s - 4, 160),    # c21
                    (118, P2R, 10 * s - 2, 80)]     # p2
            for o0, n, gg0, lim in segs:
                rows = gg0 + np.arange(n)
                vmask[j, :, o0:o0 + n] = ((rows >= 0) & (rows < lim)).astype(np.float32)
        m = dict(common)
        m["fin"] = fin
        m["fref"] = fref
        m["imgp"] = imgp
        m["vmask"] = vmask
        in_maps.append(m)
    return in_maps


def _shift_np(x, si, sj):
    # shift content down/right with zero fill; x: [B,K,H,W,2]
    return np.pad(x, ((0, 0), (0, 0), (si, 0), (sj, 0), (0, 0)))[:, :, :x.shape[2], :x.shape[3], :]


def _postprocess(results):
    K = 3
    top_idx = np.zeros((B, K, W2, W2), np.int64)
    top_val = np.zeros((B, K, W2, W2), np.float32)
    m = np.arange(W2 * W)
    valid = (m % W) < W2
    scale = np.float32(1.0) / np.float32((3.0 + EPS_PATCH) * (3.0 + EPS_PATCH))
    for b in range(B):
        tv = np.concatenate([results[4 * b + q]["topv"] for q in range(4)], 0)[:W2 * W]
        ti = np.concatenate([results[4 * b + q]["topi"] for q in range(4)], 0)[:W2 * W]
        idx78 = ti[valid][:, :K].astype(np.int64)
        v = tv[valid][:, :K].astype(np.float32) * scale
        top_idx[b] = idx78.reshape(W2, W2, K).transpose(2, 0, 1)
        top_val[b] = v.reshape(W2, W2, K).transpose(2, 0, 1)

    fw = (top_idx % W2).astype(np.float32)
    fh = (top_idx // W2).astype(np.float32)
    gx = np.arange(W2, dtype=np.float32)
    gy = np.arange(W2, dtype=np.float32)
    flow = np.stack([fw - gx[None, None, None, :], fh - gy[None, None, :, None]], -1)
    flow3 = np.pad(flow, ((0, 0), (0, 0), (0, 2), (0, 2), (0, 0)))
    off3 = np.stack([_shift_np(flow3, i, j) for i in range(3) for j in range(3)], 2)
    sim3 = np.pad(top_val, ((0, 0), (0, 0), (1, 1), (1, 1)))

    flow2 = np.repeat(np.repeat(flow3, 2, 2), 2, 3) * np.float32(2.0)
    off2 = np.stack([_shift_np(flow2, 2 * i, 2 * j) for i in range(3) for j in range(3)], 2)
    sim2 = np.repeat(np.repeat(sim3, 2, 2), 2, 3)

    flow1 = np.repeat(np.repeat(flow3, 4, 2), 4, 3) * np.float32(4.0)
    off1 = np.stack([_shift_np(flow1, 4 * i, 4 * j) for i in range(3) for j in range(3)], 2)
    sim1 = np.repeat(np.repeat(sim3, 4, 2), 4, 3)

    r1 = np.zeros((B, 64, 320, 320), np.float32)
    r2 = np.zeros((B, 128, 160, 160), np.float32)
    r3 = np.zeros((B, 256, 80, 80), np.float32)
    for c in range(8):
        b, q = c // 4, c % 4
        for j in range(JOBS):
            s = 2 * q + j
            r1[b, :, 40 * s:40 * s + 40, :] = results[c]["r1o"][j]
            r2[b, :, 20 * s:20 * s + 20, :] = results[c]["r2o"][j]
            r3[b, 0:128, 10 * s:10 * s + 10, :] = results[c]["r3o"][j, 0]
            r3[b, 128:256, 10 * s:10 * s + 10, :] = results[c]["r3o"][j, 1]

    return (flow1.astype(np.float32), flow2.astype(np.float32), flow3.astype(np.float32),
            off1.astype(np.float32), off2.astype(np.float32), off3.astype(np.float32),
            sim1, sim2, sim3, r1, r2, r3)


LAST_RUN_SECONDS = None


def kernel(dense_features1, dense_features2, img_ref_hr, vgg_params):
    import time
    global LAST_RUN_SECONDS
    if "nc" not in _CACHE:
        nc = _build_program()
        _split_excess_waits(nc)  # hardware codegen: <=1 sync wait per inst
        _CACHE["nc"] = nc
    nc = _CACHE["nc"]
    in_maps = _prep_inputs(dense_features1, dense_features2, img_ref_hr, vgg_params)
    t0 = time.time()
    res = run_bass_kernel_spmd(nc, in_maps, list(range(8)))
    LAST_RUN_SECONDS = time.time() - t0
    return _postprocess(res.results)


# revision 26
# speedup vs baseline: 1.2715x; 1.0727x over previous
"""Trainium2 Bass kernel for CorrespondenceGenerationArch.

Per-core (8 cores = 2 samples x 4 quarters), one SPMD program:
  - per-pixel L2 feature normalization on device (sum-of-squares via
    ones-matmul, column broadcast via K=1 matmul)
  - correlation GEMM [1664 x 2304] @ [2304 x 6084] as a 3-pass fp16 hi/lo
    split (hi+lo reconstructs fp32 to ~2^-22; 1 cyc/row vs fp32's 4); patch
    extraction is pure access patterns (shifted slices), exact-N tiling via
    2D rhs APs (13 x 468 = 6 patch rows x 78)
  - top-8 values+indices per output row in one DVE max/max_index pair over
    the full 6084-wide corr row in SBUF
  - VGG head (conv1_1..conv3_1) on 2 spatial 40-row strips per core:
    im2col conv1_1 (K=27), kj-pair-packed conv1_2/conv2_1 (K=128), relu+bias
    fused into PSUM eviction, host-supplied row masks for image-boundary halo
Host: input slicing/padding, weight layout prep, flow/offset/sim expansion
from the top-3 indices/values (pure index arithmetic), output assembly.
The patch-norm scales are mathematically (3+1e-5) (patches of unit-norm
pixels) and are applied to the 3 selected values on the host; top-k order is
unaffected by them up to fp noise.
"""
import sys

if "/opt/trn_rl_repo" not in sys.path:
    sys.path.insert(0, "/opt/trn_rl_repo")

import numpy as np

import concourse.bass as bass
import concourse.mybir as mybir
import concourse.tile as tile
from concourse.bass_utils import run_bass_kernel_spmd

F32 = mybir.dt.float32
U32 = mybir.dt.uint32
AF = mybir.ActivationFunctionType

# ---------------- problem constants (hardcoded) ----------------
B, C, H, W = 2, 256, 80, 80
PIX = H * W                      # 6400
W2 = H - 2                       # 78
NREF = W2 * W2                   # 6084 valid ref patches
NT, NTW = 13, 480                # N tiles (GEMM uses NTE=468 = 6 rows x 78)
NTE = 468
MC = 13                          # M chunks of 128 per core
MPC = MC * 128                   # 1664 M rows per core
FINW = 1856                      # fin slice width (1664 + 162 pad -> 1856)
FREFW = 6592                     # fref width (6400 + 162 pad -> 6592)
DKS = [di * W + dj for di in range(3) for dj in range(3)]  # patch offsets
EPS_PATCH = 1e-5

# VGG strip geometry (per job; 16 strips of 40 rows at 320-res, 2 jobs/core)
JOBS = 2
W322, W328 = 322, 328
C11R = 60          # c11 buffer rows (58 valid + slack)
CR11 = 10          # conv1_1 chunk rows
CR12 = 12          # conv1_2 chunk rows (must be even)
P1R, P1W = 30, 162   # pool1 buffer rows/width (28 valid + 2)
C21R = 28          # conv2_1 buffer rows (26 valid + 2)
C22R = 24          # conv2_2 rows (exactly valid)
P2R, P2W = 14, 82    # pool2 rows/width (12 valid + 2)

_CACHE = {}
SKIP_VGG = False
SKIP_GEMM = False
SKIP_TOPK = False
SKIP_IMCDMA = False
SKIP_EVICT = False
VGG_STAGES = 99


# ---------------- BIR post-pass: walrus accepts 1 sync-wait/instruction ----
def _split_excess_waits(nc, cap=1):
    n = 0
    for f in nc.m.functions:
        for bb in f.blocks:
            il = bb.instructions
            out = []
            changed = False
            for ins in il:
                si = getattr(ins, "sync_info", None)
                ow = list(si.on_wait) if si is not None and si.on_wait else []
                k = 0
                while len(ow) > cap:
                    chunk, ow = ow[:cap], ow[cap:]
                    out.append(mybir.InstNoOp(
                        name=f"{ins.name}_ws{k}",
                        sync_info=mybir.SyncInfo(on_wait=chunk, on_update=[]),
                        engine=ins.engine,
                        bass_nofuse=True,
                    ))
                    k += 1
                    n += 1
                if k:
                    si.on_wait = ow
                    changed = True
                out.append(ins)
            if changed:
                il[:] = out
    return n


# ---------------- device program ----------------
def _normalize(nc, tc, pool, psum, f_t, width, subtiles, ones_col, ones_row):
    """Per-pixel (column) L2-normalize f_t[2][128, width] in place."""
    row_pool = pool
    sums = row_pool.tile([1, width], F32, tag="normrow")
    for s0, sw in subtiles:
        sq = row_pool.tile([128, NTW], F32, tag="sqtmp")
        ps = psum.tile([1, NTW], F32, tag="normps")
        for ch in range(2):
            nc.vector.tensor_mul(sq[:, :sw], f_t[ch][:, s0:s0 + sw], f_t[ch][:, s0:s0 + sw])
            nc.tensor.matmul(ps[:, :sw], ones_col[:], sq[:, :sw],
                             start=(ch == 0), stop=(ch == 1))
        nc.vector.tensor_copy(sums[:, s0:s0 + sw], ps[:, :sw])
    # norm = max(sqrt(sumsq), 1e-12); r = 1/norm
    nc.scalar.activation(sums[:], sums[:], AF.Sqrt)
    nc.vector.tensor_scalar_max(sums[:], sums[:], 1.0e-12)
    nc.vector.reciprocal(sums[:], sums[:])
    for s0, sw in subtiles:
        bc = psum.tile([128, NTW], F32, tag="bcps")
        nc.tensor.matmul(bc[:, :sw], ones_row[:], sums[:, s0:s0 + sw],
                         start=True, stop=True)
        for ch in range(2):
            nc.vector.tensor_mul(f_t[ch][:, s0:s0 + sw], f_t[ch][:, s0:s0 + sw], bc[:, :sw])


def _subtiles(width):
    out = []
    s = 0
    while s < width:
        out.append((s, min(NTW, width - s)))
        s += NTW
    return out


def _build_program():
    nc = bass.Bass()

    fin_d = nc.dram_tensor("fin", [C, FINW], F32, kind="ExternalInput")
    fref_d = nc.dram_tensor("fref", [C, FREFW], F32, kind="ExternalInput")
    imgp_d = nc.dram_tensor("imgp", [JOBS, 3, C11R, W328], F32, kind="ExternalInput")
    w1c_d = nc.dram_tensor("w1c", [27, 64], F32, kind="ExternalInput")
    b1_d = nc.dram_tensor("b1", [64, 1], F32, kind="ExternalInput")
    w12p_d = nc.dram_tensor("w12p", [128, 3, 64], F32, kind="ExternalInput")
    w12l_d = nc.dram_tensor("w12l", [64, 3, 64], F32, kind="ExternalInput")
    b12_d = nc.dram_tensor("b12", [64, 1], F32, kind="ExternalInput")
    w21p_d = nc.dram_tensor("w21p", [128, 3, 128], F32, kind="ExternalInput")
    w21l_d = nc.dram_tensor("w21l", [64, 3, 128], F32, kind="ExternalInput")
    b21_d = nc.dram_tensor("b21", [128, 1], F32, kind="ExternalInput")
    w22_d = nc.dram_tensor("w22", [128, 9, 128], F32, kind="ExternalInput")
    b22_d = nc.dram_tensor("b22", [128, 1], F32, kind="ExternalInput")
    w31_d = nc.dram_tensor("w31", [128, 18, 128], F32, kind="ExternalInput")
    b31_d = nc.dram_tensor("b31", [128, 2], F32, kind="ExternalInput")
    # per-level out-of-image row masks: segments c11[0:60] p1[60:90] c21[90:118] p2[118:132]
    vmask_d = nc.dram_tensor("vmask", [JOBS, 128, 132], F32, kind="ExternalInput")

    topv_d = nc.dram_tensor("topv", [MPC, 8], F32, kind="ExternalOutput")
    topi_d = nc.dram_tensor("topi", [MPC, 8], U32, kind="ExternalOutput")
    r1_d = nc.dram_tensor("r1o", [JOBS, 64, 40, 320], F32, kind="ExternalOutput")
    r2_d = nc.dram_tensor("r2o", [JOBS, 128, 20, 160], F32, kind="ExternalOutput")
    r3_d = nc.dram_tensor("r3o", [JOBS, 2, 128, 10, 80], F32, kind="ExternalOutput")

    with tile.TileContext(nc) as tc:
        # ---------------- phase 1: correlation + topk ----------------
        F16 = mybir.dt.float16
        with tc.tile_pool(name="feat", bufs=1) as feat, \
             tc.tile_pool(name="small", bufs=2) as small, \
             tc.tile_pool(name="gpsum", bufs=2, space="PSUM") as gpsum:

            # fp16 hi/lo split of the normalized features (hi+lo == fp32 value
            # to ~2^-22): 3 fp16 matmul passes run at 1 cyc/row vs fp32's 4.
            fin_h = [feat.tile([128, FINW], F16, tag=f"finh{ch}", name=f"finh{ch}") for ch in range(2)]
            fin_l = [feat.tile([128, FINW], F16, tag=f"finl{ch}", name=f"finl{ch}") for ch in range(2)]
            fref_h = [feat.tile([128, FREFW], F16, tag=f"frefh{ch}", name=f"frefh{ch}") for ch in range(2)]
            fref_l = [feat.tile([128, FREFW], F16, tag=f"frefl{ch}", name=f"frefl{ch}") for ch in range(2)]

            with tc.tile_pool(name="rawf", bufs=1) as rawf:
                ones_col = rawf.tile([128, 1], F32)
                nc.vector.memset(ones_col[:], 1.0)
                ones_row = rawf.tile([1, 128], F32)
                nc.vector.memset(ones_row[:], 1.0)
                fin_t = [rawf.tile([128, FINW], F32, tag=f"fin{ch}", name=f"fin{ch}") for ch in range(2)]
                fref_t = [rawf.tile([128, FREFW], F32, tag=f"fref{ch}", name=f"fref{ch}") for ch in range(2)]
                for ch in range(2):
                    for s0, sw in _subtiles(FINW):
                        nc.sync.dma_start(fin_t[ch][:, s0:s0 + sw],
                                          fin_d[ch * 128:(ch + 1) * 128, s0:s0 + sw])
                    for s0, sw in _subtiles(FREFW):
                        nc.sync.dma_start(fref_t[ch][:, s0:s0 + sw],
                                          fref_d[ch * 128:(ch + 1) * 128, s0:s0 + sw])

                _normalize(nc, tc, small, gpsum, fin_t, FINW, _subtiles(FINW), ones_col, ones_row)
                _normalize(nc, tc, small, gpsum, fref_t, FREFW, _subtiles(FREFW), ones_col, ones_row)

                for ch in range(2):
                    for f32t, h, lo, width in ((fin_t[ch], fin_h[ch], fin_l[ch], FINW),
                                               (fref_t[ch], fref_h[ch], fref_l[ch], FREFW)):
                        for s0, sw in _subtiles(width):
                            tmp = small.tile([128, NTW], F32, tag="sqtmp", name="spl")
                            nc.vector.tensor_copy(h[:, s0:s0 + sw], f32t[:, s0:s0 + sw])
                            nc.vector.tensor_copy(tmp[:, :sw], h[:, s0:s0 + sw])
                            nc.vector.tensor_sub(tmp[:, :sw], f32t[:, s0:s0 + sw], tmp[:, :sw])
                            nc.vector.tensor_copy(lo[:, s0:s0 + sw], tmp[:, :sw])

            corr_cm = tc.tile_pool(name="corr", bufs=2)
            corrp = corr_cm.__enter__()
            for mc in range(MC if not SKIP_GEMM else 0):
                m0 = mc * 128
                corr = corrp.tile([128, NREF], F32, tag="corr")
                for j in range(NT):
                    n0pix = j * 6 * W
                    pt = gpsum.tile([128, NTE], F32, tag="gemm")
                    n = 0
                    for fa, fb in ((fin_h, fref_h), (fin_h, fref_l), (fin_l, fref_h)):
                        for kt in range(18):
                            ch, dk = kt // 9, DKS[kt % 9]
                            n += 1
                            rhs = fb[ch][:, n0pix + dk:n0pix + dk + 6 * W] \
                                .rearrange("p (r c) -> p r c", c=W)[:, :, 0:W2]
                            nc.tensor.matmul(
                                pt[:],
                                fa[ch][:, m0 + dk:m0 + dk + 128],
                                rhs,
                                start=(n == 1), stop=(n == 54))
                    nc.vector.tensor_copy(corr[:, j * NTE:(j + 1) * NTE], pt[:])
                tv = small.tile([128, 8], F32, tag="tv")
                ti = small.tile([128, 8], U32, tag="ti")
                nc.vector.max(out=tv[:], in_=corr[:])
                nc.vector.max_index(out=ti[:], in_max=tv[:], in_values=corr[:])
                nc.sync.dma_start(topv_d[m0:m0 + 128, :], tv[:])
                nc.sync.dma_start(topi_d[m0:m0 + 128, :], ti[:])
            corr_cm.__exit__(None, None, None)

        # ---------------- phase 2: VGG head ----------------
        if SKIP_VGG:
            _ = 0
        with tc.tile_pool(name="vw", bufs=1) as vw, \
             tc.tile_pool(name="vbuf", bufs=1) as vbuf, \
             tc.tile_pool(name="vchunk", bufs=2) as vchunk, \
             tc.tile_pool(name="vpsum", bufs=4, space="PSUM") as vpsum:

            w1c_t = vw.tile([27, 64], F32)
            nc.sync.dma_start(w1c_t[:], w1c_d[:])
            w12p_t = vw.tile([128, 3, 64], F32)
            nc.sync.dma_start(w12p_t[:], w12p_d[:])
            w12l_t = vw.tile([64, 3, 64], F32)
            nc.sync.dma_start(w12l_t[:], w12l_d[:])
            w21p_t = vw.tile([128, 3, 128], F32)
            nc.sync.dma_start(w21p_t[:], w21p_d[:])
            w21l_t = vw.tile([64, 3, 128], F32)
            nc.sync.dma_start(w21l_t[:], w21l_d[:])
            w22_t = vw.tile([128, 9, 128], F32)
            nc.sync.dma_start(w22_t[:], w22_d[:])
            w31_t = vw.tile([128, 18, 128], F32)
            nc.sync.dma_start(w31_t[:], w31_d[:])
            b1_t = vw.tile([64, 1], F32)
            nc.sync.dma_start(b1_t[:], b1_d[:])
            b12_t = vw.tile([64, 1], F32)
            nc.sync.dma_start(b12_t[:], b12_d[:])
            b21_t = vw.tile([128, 1], F32)
            nc.sync.dma_start(b21_t[:], b21_d[:])
            b22_t = vw.tile([128, 1], F32)
            nc.sync.dma_start(b22_t[:], b22_d[:])
            b31_t = vw.tile([128, 2], F32)
            nc.sync.dma_start(b31_t[:], b31_d[:])
            vmask_t = vw.tile([128, JOBS, 132], F32)
            nc.sync.dma_start(vmask_t[:], vmask_d.rearrange("j p s -> p j s"))

            for jj in range(JOBS if not SKIP_VGG else 0):
                # ---- buffers (flat [128, rows*width]) ----
                c11 = vbuf.tile([128, C11R * W322], F32, tag="c11")
                p1 = vbuf.tile([128, P1R * P1W], F32, tag="p1")
                c21 = vbuf.tile([128, C21R * P1W], F32, tag="c21")
                c22 = vbuf.tile([128, C22R * P1W], F32, tag="c22")
                p2 = vbuf.tile([128, P2R * P2W], F32, tag="p2")
                c31 = vbuf.tile([128, 2, 10 * 80], F32, tag="c31")
                nc.gpsimd.memset(c11[:], 0.0)
                nc.gpsimd.memset(p1[:], 0.0)
                nc.gpsimd.memset(c21[:], 0.0)
                nc.gpsimd.memset(p2[:], 0.0)

                # ---- conv1_1: im2col K=27; writes c11 rows [1, 59) ----
                nrows = 58 if VGG_STAGES >= 1 else 0
                r = 0
                while r < nrows:
                    cr = min(CR11, nrows - r)   # chunk c11 rows [1+r, 1+r+cr)
                    imc = vchunk.tile([27, CR11 * W322], F32, tag="vsA")
                    for ki in range(3):
                        for kj in range(3):
                            p0 = (ki * 3 + kj) * 3
                            dst = imc[p0:p0 + 3, :cr * W322].rearrange(
                                "p (r x) -> p r x", x=W322)
                            nc.sync.dma_start(
                                dst, imgp_d[jj, :, r + ki:r + ki + cr, kj:kj + W322])
                    nflat = cr * W322
                    for t0, tw in _subtiles(nflat):
                        ps = vpsum.tile([64, NTW], F32, tag="vps")
                        nc.tensor.matmul(ps[:, :tw], w1c_t[:], imc[:, t0:t0 + tw],
                                         start=True, stop=True)
                        nc.scalar.activation(
                            c11[0:64, (1 + r) * W322 + t0:(1 + r) * W322 + t0 + tw],
                            ps[:, :tw], AF.Relu, bias=b1_t[:])
                    r += cr
                # zero the wrap border pairs (x=321 of row r / x=0 of row r+1)
                c11v = c11[0:64].rearrange("p (r x) -> p r x", x=W322)
                nc.vector.memset(c11v[:, 0:C11R - 1, W322 - 1:W322], 0.0)
                nc.vector.memset(c11v[:, 1:C11R, 0:1], 0.0)
                # zero out-of-image halo rows (only boundary rows can be masked)
                for r0, r1 in ((0, 12), (48, 60)):
                    nc.vector.tensor_mul(
                        c11v[:, r0:r1, :], c11v[:, r0:r1, :],
                        vmask_t[0:64, jj, r0:r1].unsqueeze(2).broadcast_to((64, r1 - r0, W322)))
                # r1 out: rows 10..50, cols 1..321
                nc.sync.dma_start(r1_d[jj], c11v[:, 10:50, 1:321])
                # pairing: partitions 64:128 = col+1
                nc.vector.tensor_copy(c11[64:128, 0:C11R * W322 - 1],
                                      c11[0:64, 1:C11R * W322])
                nc.vector.memset(c11[64:128, C11R * W322 - 1:C11R * W322], 0.0)

                # ---- conv1_2 (+pool1): output rows gr in [0,56) ----
                gr = 0 if VGG_STAGES >= 3 else 99
                pu = 1  # pool1 write row
                while gr < 56:
                    cr = min(CR12, 56 - gr)
                    cc = vchunk.tile([64, CR12 * W322], F32, tag="vsA")
                    nflat = cr * W322
                    for t0, tw in _subtiles(nflat):
                        ps = vpsum.tile([64, NTW], F32, tag="vps")
                        for ki in range(3):
                            base = (gr + 1 + ki) * W322 - 1 + t0
                            nc.tensor.matmul(ps[:, :tw], w12p_t[:, ki, :],
                                             c11[:, base:base + tw],
                                             start=(ki == 0), stop=False)
                            nc.tensor.matmul(ps[:, :tw], w12l_t[:, ki, :],
                                             c11[0:64, base + 2:base + 2 + tw],
                                             start=False, stop=(ki == 2))
                        nc.scalar.activation(cc[:, t0:t0 + tw], ps[:, :tw],
                                             AF.Relu, bias=b12_t[:])
                    # pool 2x2 -> p1 rows [pu, pu+cr/2), cols [1,161)
                    ccv2 = cc[:, :nflat].rearrange("p (r two x) -> p r two x",
                                                   two=2, x=W322)
                    rm = vchunk.tile([64, (CR12 // 2) * W322], F32, tag="vsB")
                    rmv = rm[:, :(cr // 2) * W322].rearrange("p (r x) -> p r x", x=W322)
                    nc.vector.tensor_max(rmv[:], ccv2[:, :, 0, :], ccv2[:, :, 1, :])
                    rmp = rmv[:, :, 1:321].rearrange("p r (xh two) -> p r xh two", two=2)
                    p1v = p1[0:64].rearrange("p (r x) -> p r x", x=P1W)
                    nc.vector.tensor_max(p1v[:, pu:pu + cr // 2, 1:161],
                                         rmp[:, :, :, 0], rmp[:, :, :, 1])
                    gr += cr
                    pu += cr // 2
                # mask out-of-image pool1 rows, then pairing
                p1mv = p1[0:64].rearrange("p (r x) -> p r x", x=P1W)
                for r0, r1 in ((0, 6), (24, 30)):
                    nc.vector.tensor_mul(
                        p1mv[:, r0:r1, :], p1mv[:, r0:r1, :],
                        vmask_t[0:64, jj, 60 + r0:60 + r1].unsqueeze(2).broadcast_to((64, r1 - r0, P1W)))
                nc.vector.tensor_copy(p1[64:128, 0:P1R * P1W - 1],
                                      p1[0:64, 1:P1R * P1W])
                nc.vector.memset(p1[64:128, P1R * P1W - 1:P1R * P1W], 0.0)

                # ---- conv2_1: output c21 rows [1,27), flat n over 26*162 ----
                nflat = 26 * P1W if VGG_STAGES >= 4 else 0
                for t0, tw in _subtiles(nflat):
                    ps = vpsum.tile([128, NTW], F32, tag="vps")
                    for ki in range(3):
                        base = (1 + ki) * P1W - 1 + t0
                        nc.tensor.matmul(ps[:, :tw], w21p_t[:, ki, :],
                                         p1[:, base:base + tw],
                                         start=(ki == 0), stop=False)
                        nc.tensor.matmul(ps[:, :tw], w21l_t[:, ki, :],
                                         p1[0:64, base + 2:base + 2 + tw],
                                         start=False, stop=(ki == 2))
                    nc.scalar.activation(c21[:, P1W + t0:P1W + t0 + tw], ps[:, :tw],
                                         AF.Relu, bias=b21_t[:])
                c21v = c21[:].rearrange("p (r x) -> p r x", x=P1W)
                nc.vector.memset(c21v[:, 0:C21R - 1, P1W - 1:P1W], 0.0)
                nc.vector.memset(c21v[:, 1:C21R, 0:1], 0.0)
                for r0, r1 in ((0, 4), (24, 28)):
                    nc.vector.tensor_mul(
                        c21v[:, r0:r1, :], c21v[:, r0:r1, :],
                        vmask_t[:, jj, 90 + r0:90 + r1].unsqueeze(2).broadcast_to((128, r1 - r0, P1W)))
                # r2 out: c21 rows 4..24, cols 1..161
                nc.sync.dma_start(r2_d[jj], c21v[:, 4:24, 1:161])

                # ---- conv2_2: output c22 rows [0,24) == valid ----
                nflat = C22R * P1W if VGG_STAGES >= 5 else 0
                for t0, tw in _subtiles(nflat):
                    ps = vpsum.tile([128, NTW], F32, tag="vps")
                    for kt in range(9):
                        ki, kj = kt // 3, kt % 3
                        base = (1 + ki) * P1W - 1 + kj + t0
                        nc.tensor.matmul(ps[:, :tw], w22_t[:, kt, :],
                                         c21[:, base:base + tw],
                                         start=(kt == 0), stop=(kt == 8))
                    nc.scalar.activation(c22[:, t0:t0 + tw], ps[:, :tw],
                                         AF.Relu, bias=b22_t[:])
                # pool2 -> p2 rows [1,13), cols [1,81)
                c22v2 = c22[:].rearrange("p (r two x) -> p r two x", two=2, x=P1W)
                rm2 = vchunk.tile([128, 12 * P1W], F32, tag="vsB")
                rm2v = rm2[:].rearrange("p (r x) -> p r x", x=P1W)
                nc.vector.tensor_max(rm2v[:], c22v2[:, :, 0, :], c22v2[:, :, 1, :])
                rm2p = rm2v[:, :, 1:161].rearrange("p r (xh two) -> p r xh two", two=2)
                p2v = p2[:].rearrange("p (r x) -> p r x", x=P2W)
                nc.vector.tensor_max(p2v[:, 1:13, 1:81],
                                     rm2p[:, :, :, 0], rm2p[:, :, :, 1])
                for r0, r1 in ((0, 2), (12, 14)):
                    nc.vector.tensor_mul(
                        p2v[:, r0:r1, :], p2v[:, r0:r1, :],
                        vmask_t[:, jj, 118 + r0:118 + r1].unsqueeze(2).broadcast_to((128, r1 - r0, P2W)))

                # ---- conv3_1: out rows [0,10), N = 10*82 in 2 tiles of 5 rows ----
                for half in range(2 if VGG_STAGES >= 6 else 0):
                    for u0 in (0, 5):
                        ps = vpsum.tile([128, 5 * P2W], F32, tag="vps")
                        for kt in range(9):
                            ki, kj = kt // 3, kt % 3
                            base = (u0 + 1 + ki) * P2W - 1 + kj
                            nc.tensor.matmul(ps[:], w31_t[:, 2 * kt + half, :],
                                             p2[:, base:base + 5 * P2W],
                                             start=(kt == 0), stop=(kt == 8))
                        psv = ps[:].rearrange("p (r x) -> p r x", x=P2W)
                        outv = c31[:, half].rearrange("p (r x) -> p r x", x=80)
                        nc.scalar.activation(outv[:, u0:u0 + 5, :],
                                             psv[:, :, 1:81], AF.Relu,
                                             bias=b31_t[:, half:half + 1])
                    nc.sync.dma_start(
                        r3_d[jj, half],
                        c31[:, half].rearrange("p (r x) -> p r x", x=80))

    return nc


# ---------------- host-side helpers ----------------
def _prep_inputs(dense_features1, dense_features2, img_ref_hr, vgg_params):
    d1 = np.ascontiguousarray(dense_features1, np.float32).reshape(B, C, PIX)
    d2 = np.ascontiguousarray(dense_features2, np.float32).reshape(B, C, PIX)
    img = np.ascontiguousarray(img_ref_hr, np.float32)

    mean = np.array([0.485, 0.456, 0.406], np.float32).reshape(3, 1, 1)
    std = np.array([0.229, 0.224, 0.225], np.float32).reshape(3, 1, 1)
    w11 = np.asarray(vgg_params["w1_1"], np.float32)
    b1f = np.asarray(vgg_params["b1_1"], np.float32)
    # image is normalized on host (padding must be zero in normalized domain)
    img = (img - mean[None]) / std[None]
    # layout [ (ki*3+kj)*3 + c, o ]
    w1c = np.ascontiguousarray(
        w11.transpose(2, 3, 1, 0).reshape(27, 64))

    def pair_weights(wkey, cin):
        wmat = np.asarray(vgg_params[wkey], np.float32)  # [o, i, 3, 3]
        cout = wmat.shape[0]
        wp = np.zeros((128, 3, cout), np.float32)
        wl = np.zeros((64, 3, cout), np.float32)
        for ki in range(3):
            wp[:cin, ki] = wmat[:, :, ki, 0].T
            wp[64:64 + cin, ki] = wmat[:, :, ki, 1].T
            wl[:cin, ki] = wmat[:, :, ki, 2].T
        return np.ascontiguousarray(wp), np.ascontiguousarray(wl)

    w12p, w12l = pair_weights("w1_2", 64)
    w21p, w21l = pair_weights("w2_1", 64)
    w22m = np.asarray(vgg_params["w2_2"], np.float32)
    w22 = np.ascontiguousarray(
        w22m.transpose(2, 3, 1, 0).reshape(9, 128, 128).transpose(1, 0, 2))
    w31m = np.asarray(vgg_params["w3_1"], np.float32)  # [256,128,3,3]
    w31 = np.zeros((128, 18, 128), np.float32)
    for kt in range(9):
        ki, kj = kt // 3, kt % 3
        w31[:, 2 * kt + 0] = w31m[0:128, :, ki, kj].T
        w31[:, 2 * kt + 1] = w31m[128:256, :, ki, kj].T

    common = {
        "w1c": w1c, "b1": b1f.reshape(64, 1).astype(np.float32),
        "w12p": w12p, "w12l": w12l,
        "b12": np.asarray(vgg_params["b1_2"], np.float32).reshape(64, 1),
        "w21p": w21p, "w21l": w21l,
        "b21": np.asarray(vgg_params["b2_1"], np.float32).reshape(128, 1),
        "w22": w22,
        "b22": np.asarray(vgg_params["b2_2"], np.float32).reshape(128, 1),
        "w31": np.ascontiguousarray(w31),
        "b31": np.ascontiguousarray(
            np.asarray(vgg_params["b3_1"], np.float32).reshape(2, 128).T),
    }

    in_maps = []
    for c in range(8):
        b, q = c // 4, c % 4
        fin = np.zeros((C, FINW), np.float32)
        lo = q * MPC
        hi = min(PIX, lo + FINW)
        fin[:, :hi - lo] = d1[b, :, lo:hi]
        fref = np.zeros((C, FREFW), np.float32)
        fref[:, :PIX] = d2[b]
        imgp = np.zeros((JOBS, 3, C11R, W328), np.float32)
        vmask = np.zeros((JOBS, 128, 132), np.float32)
        for j in range(JOBS):
            s = 2 * q + j
            g0 = 40 * s - 10       # imgp row 0 <-> image row g0
            r0 = max(0, -g0)
            r1 = min(C11R, 320 - g0)
            imgp[j, :, r0:r1, 2:322] = img[b, :, g0 + r0:g0 + r1, :]
            segs = [(0, C11R, 40 * s - 10, 320),    # c11: row lr <-> 40s-10+lr
                    (60, P1R, 20 * s - 5, 160),     # p1
                    (90, C21R, 20 * s - 4, 160),    # c21
                    (118, P2R, 10 * s - 2, 80)]     # p2
            for o0, n, gg0, lim in segs:
                rows = gg0 + np.arange(n)
                vmask[j, :, o0:o0 + n] = ((rows >= 0) & (rows < lim)).astype(np.float32)
        m = dict(common)
        m["fin"] = fin
        m["fref"] = fref
        m["imgp"] = imgp
        m["vmask"] = vmask
        in_maps.append(m)
    return in_maps


def _shift_np(x, si, sj):
    # shift content down/right with zero fill; x: [B,K,H,W,2]
    return np.pad(x, ((0, 0), (0, 0), (si, 0), (sj, 0), (0, 0)))[:, :, :x.shape[2], :x.shape[3], :]


def _postprocess(results):
    K = 3
    top_idx = np.zeros((B, K, W2, W2), np.int64)
    top_val = np.zeros((B, K, W2, W2), np.float32)
    m = np.arange(W2 * W)
    valid = (m % W) < W2
    scale = np.float32(1.0) / np.float32((3.0 + EPS_PATCH) * (3.0 + EPS_PATCH))
    for b in range(B):
        tv = np.concatenate([results[4 * b + q]["topv"] for q in range(4)], 0)[:W2 * W]
        ti = np.concatenate([results[4 * b + q]["topi"] for q in range(4)], 0)[:W2 * W]
        idx78 = ti[valid][:, :K].astype(np.int64)
        v = tv[valid][:, :K].astype(np.float32) * scale
        top_idx[b] = idx78.reshape(W2, W2, K).transpose(2, 0, 1)
        top_val[b] = v.reshape(W2, W2, K).transpose(2, 0, 1)

    fw = (top_idx % W2).astype(np.float32)
    fh = (top_idx // W2).astype(np.float32)
    gx = np.arange(W2, dtype=np.float32)
    gy = np.arange(W2, dtype=np.float32)
    flow = np.stack([fw - gx[None, None, None, :], fh - gy[None, None, :, None]], -1)
    flow3 = np.pad(flow, ((0, 0), (0, 0), (0, 2), (0, 2), (0, 0)))
    off3 = np.stack([_shift_np(flow3, i, j) for i in range(3) for j in range(3)], 2)
    sim3 = np.pad(top_val, ((0, 0), (0, 0), (1, 1), (1, 1)))

    flow2 = np.repeat(np.repeat(flow3, 2, 2), 2, 3) * np.float32(2.0)
    off2 = np.stack([_shift_np(flow2, 2 * i, 2 * j) for i in range(3) for j in range(3)], 2)
    sim2 = np.repeat(np.repeat(sim3, 2, 2), 2, 3)

    flow1 = np.repeat(np.repeat(flow3, 4, 2), 4, 3) * np.float32(4.0)
    off1 = np.stack([_shift_np(flow1, 4 * i, 4 * j) for i in range(3) for j in range(3)], 2)
    sim1 = np.repeat(np.repeat(sim3, 4, 2), 4, 3)

    r1 = np.zeros((B, 64, 320, 320), np.float32)
    r2 = np.zeros((B, 128, 160, 160), np.float32)
    r3 = np.zeros((B, 256, 80, 80), np.float32)
    for c in range(8):
        b, q = c // 4, c % 4
        for j in range(JOBS):
            s = 2 * q + j
            r1[b, :, 40 * s:40 * s + 40, :] = results[c]["r1o"][j]
            r2[b, :, 20 * s:20 * s + 20, :] = results[c]["r2o"][j]
            r3[b, 0:128, 10 * s:10 * s + 10, :] = results[c]["r3o"][j, 0]
            r3[b, 128:256, 10 * s:10 * s + 10, :] = results[c]["r3o"][j, 1]

    return (flow1.astype(np.float32), flow2.astype(np.float32), flow3.astype(np.float32),
            off1.astype(np.float32), off2.astype(np.float32), off3.astype(np.float32),
            sim1, sim2, sim3, r1, r2, r3)


LAST_RUN_SECONDS = None


def kernel(dense_features1, dense_features2, img_ref_hr, vgg_params):
    import time
    global LAST_RUN_SECONDS
    if "nc" not in _CACHE:
        nc = _build_program()
        _split_excess_waits(nc)  # hardware codegen: <=1 sync wait per inst
        _CACHE["nc"] = nc
    nc = _CACHE["nc"]
    in_maps = _prep_inputs(dense_features1, dense_features2, img_ref_hr, vgg_params)
    t0 = time.time()
    res = run_bass_kernel_spmd(nc, in_maps, list(range(8)))
    LAST_RUN_SECONDS = time.time() - t0
    return _postprocess(res.results)


# revision 30
# speedup vs baseline: 1.3039x; 1.0255x over previous
"""Trainium2 Bass kernel for CorrespondenceGenerationArch.

Per-core (8 cores = 2 samples x 4 quarters), one SPMD program:
  - per-pixel L2 feature normalization on device (sum-of-squares via
    ones-matmul, column broadcast via K=1 matmul)
  - correlation GEMM [1664 x 2304] @ [2304 x 6084] as a 3-pass fp16 hi/lo
    split (hi+lo reconstructs fp32 to ~2^-22; 1 cyc/row vs fp32's 4); patch
    extraction is pure access patterns (shifted slices), exact-N tiling via
    2D rhs APs (13 x 468 = 6 patch rows x 78)
  - top-8 values+indices per output row in one DVE max/max_index pair over
    the full 6084-wide corr row in SBUF
  - VGG head (conv1_1..conv3_1) on 2 spatial 40-row strips per core:
    im2col conv1_1 (K=27), kj-pair-packed conv1_2/conv2_1 (K=128), relu+bias
    fused into PSUM eviction, host-supplied row masks for image-boundary halo
Host: input slicing/padding, weight layout prep, flow/offset/sim expansion
from the top-3 indices/values (pure index arithmetic), output assembly.
The patch-norm scales are mathematically (3+1e-5) (patches of unit-norm
pixels) and are applied to the 3 selected values on the host; top-k order is
unaffected by them up to fp noise.
"""
import sys

if "/opt/trn_rl_repo" not in sys.path:
    sys.path.insert(0, "/opt/trn_rl_repo")

import numpy as np

import concourse.bass as bass
import concourse.mybir as mybir
import concourse.tile as tile
from concourse.bass_utils import run_bass_kernel_spmd

F32 = mybir.dt.float32
U32 = mybir.dt.uint32
AF = mybir.ActivationFunctionType

# ---------------- problem constants (hardcoded) ----------------
B, C, H, W = 2, 256, 80, 80
PIX = H * W                      # 6400
W2 = H - 2                       # 78
NREF = W2 * W2                   # 6084 valid ref patches
NT, NTW = 13, 480                # N tiles (GEMM uses NTE=468 = 6 rows x 78)
NTE = 468
MC = 13                          # M chunks of 128 per core
MPC = MC * 128                   # 1664 M rows per core
FINW = 1856                      # fin slice width (1664 + 162 pad -> 1856)
FREFW = 6592                     # fref width (6400 + 162 pad -> 6592)
DKS = [di * W + dj for di in range(3) for dj in range(3)]  # patch offsets
EPS_PATCH = 1e-5

# VGG strip geometry (per job; 16 strips of 40 rows at 320-res, 2 jobs/core)
JOBS = 2
W322, W328 = 322, 328
C11R = 60          # c11 buffer rows (58 valid + slack)
CR11 = 8           # conv1_1 chunk rows
CR12 = 8           # conv1_2 chunk rows (must be even)
P1R, P1W = 30, 162   # pool1 buffer rows/width (28 valid + 2)
C21R = 28          # conv2_1 buffer rows (26 valid + 2)
C22R = 24          # conv2_2 rows (exactly valid)
P2R, P2W = 14, 82    # pool2 rows/width (12 valid + 2)

_CACHE = {}
SKIP_VGG = False
SKIP_GEMM = False
SKIP_TOPK = False
SKIP_IMCDMA = False
SKIP_EVICT = False
VGG_STAGES = 99


# ---------------- BIR post-pass: walrus accepts 1 sync-wait/instruction ----
def _split_excess_waits(nc, cap=1):
    n = 0
    for f in nc.m.functions:
        for bb in f.blocks:
            il = bb.instructions
            out = []
            changed = False
            for ins in il:
                si = getattr(ins, "sync_info", None)
                ow = list(si.on_wait) if si is not None and si.on_wait else []
                k = 0
                while len(ow) > cap:
                    chunk, ow = ow[:cap], ow[cap:]
                    out.append(mybir.InstNoOp(
                        name=f"{ins.name}_ws{k}",
                        sync_info=mybir.SyncInfo(on_wait=chunk, on_update=[]),
                        engine=ins.engine,
                        bass_nofuse=True,
                    ))
                    k += 1
                    n += 1
                if k:
                    si.on_wait = ow
                    changed = True
                out.append(ins)
            if changed:
                il[:] = out
    return n


# ---------------- device program ----------------
def _normalize(nc, tc, pool, psum, f_t, width, subtiles, ones_col, ones_row):
    """Per-pixel (column) L2-normalize f_t[2][128, width] in place."""
    row_pool = pool
    sums = row_pool.tile([1, width], F32, tag="normrow")
    for s0, sw in subtiles:
        sq = row_pool.tile([128, NTW], F32, tag="sqtmp")
        ps = psum.tile([1, NTW], F32, tag="normps")
        for ch in range(2):
            nc.vector.tensor_mul(sq[:, :sw], f_t[ch][:, s0:s0 + sw], f_t[ch][:, s0:s0 + sw])
            nc.tensor.matmul(ps[:, :sw], ones_col[:], sq[:, :sw],
                             start=(ch == 0), stop=(ch == 1))
        nc.vector.tensor_copy(sums[:, s0:s0 + sw], ps[:, :sw])
    # norm = max(sqrt(sumsq), 1e-12); r = 1/norm
    nc.scalar.activation(sums[:], sums[:], AF.Sqrt)
    nc.vector.tensor_scalar_max(sums[:], sums[:], 1.0e-12)
    nc.vector.reciprocal(sums[:], sums[:])
    for s0, sw in subtiles:
        bc = psum.tile([128, NTW], F32, tag="bcps")
        nc.tensor.matmul(bc[:, :sw], ones_row[:], sums[:, s0:s0 + sw],
                         start=True, stop=True)
        for ch in range(2):
            nc.vector.tensor_mul(f_t[ch][:, s0:s0 + sw], f_t[ch][:, s0:s0 + sw], bc[:, :sw])


def _subtiles(width):
    out = []
    s = 0
    while s < width:
        out.append((s, min(NTW, width - s)))
        s += NTW
    return out


def _build_program():
    nc = bass.Bass()

    fin_d = nc.dram_tensor("fin", [C, FINW], F32, kind="ExternalInput")
    fref_d = nc.dram_tensor("fref", [C, FREFW], F32, kind="ExternalInput")
    imgp_d = nc.dram_tensor("imgp", [JOBS, 3, C11R, W328], F32, kind="ExternalInput")
    w1c_d = nc.dram_tensor("w1c", [27, 64], F32, kind="ExternalInput")
    b1_d = nc.dram_tensor("b1", [64, 1], F32, kind="ExternalInput")
    w12p_d = nc.dram_tensor("w12p", [128, 3, 64], F32, kind="ExternalInput")
    w12l_d = nc.dram_tensor("w12l", [64, 3, 64], F32, kind="ExternalInput")
    b12_d = nc.dram_tensor("b12", [64, 1], F32, kind="ExternalInput")
    w21p_d = nc.dram_tensor("w21p", [128, 3, 128], F32, kind="ExternalInput")
    w21l_d = nc.dram_tensor("w21l", [64, 3, 128], F32, kind="ExternalInput")
    b21_d = nc.dram_tensor("b21", [128, 1], F32, kind="ExternalInput")
    w22_d = nc.dram_tensor("w22", [128, 9, 128], F32, kind="ExternalInput")
    b22_d = nc.dram_tensor("b22", [128, 1], F32, kind="ExternalInput")
    w31_d = nc.dram_tensor("w31", [128, 18, 128], F32, kind="ExternalInput")
    b31_d = nc.dram_tensor("b31", [128, 2], F32, kind="ExternalInput")
    # per-level out-of-image row masks: segments c11[0:60] p1[60:90] c21[90:118] p2[118:132]
    vmask_d = nc.dram_tensor("vmask", [JOBS, 128, 132], F32, kind="ExternalInput")

    topv_d = nc.dram_tensor("topv", [MPC, 8], F32, kind="ExternalOutput")
    topi_d = nc.dram_tensor("topi", [MPC, 8], U32, kind="ExternalOutput")
    r1_d = nc.dram_tensor("r1o", [JOBS, 64, 40, 320], F32, kind="ExternalOutput")
    r2_d = nc.dram_tensor("r2o", [JOBS, 128, 20, 160], F32, kind="ExternalOutput")
    r3_d = nc.dram_tensor("r3o", [JOBS, 2, 128, 10, 80], F32, kind="ExternalOutput")

    with tile.TileContext(nc) as tc:
        # ---------------- phase 1: correlation + topk ----------------
        F16 = mybir.dt.float16
        with tc.tile_pool(name="feat", bufs=1) as feat, \
             tc.tile_pool(name="small", bufs=2) as small, \
             tc.tile_pool(name="gpsum", bufs=2, space="PSUM") as gpsum:

            # fp16 hi/lo split of the normalized features (hi+lo == fp32 value
            # to ~2^-22): 3 fp16 matmul passes run at 1 cyc/row vs fp32's 4.
            fin_h = [feat.tile([128, FINW], F16, tag=f"finh{ch}", name=f"finh{ch}") for ch in range(2)]
            fin_l = [feat.tile([128, FINW], F16, tag=f"finl{ch}", name=f"finl{ch}") for ch in range(2)]
            fref_h = [feat.tile([128, FREFW], F16, tag=f"frefh{ch}", name=f"frefh{ch}") for ch in range(2)]
            fref_l = [feat.tile([128, FREFW], F16, tag=f"frefl{ch}", name=f"frefl{ch}") for ch in range(2)]

            with tc.tile_pool(name="rawf", bufs=1) as rawf:
                ones_col = rawf.tile([128, 1], F32)
                nc.vector.memset(ones_col[:], 1.0)
                ones_row = rawf.tile([1, 128], F32)
                nc.vector.memset(ones_row[:], 1.0)
                fin_t = [rawf.tile([128, FINW], F32, tag=f"fin{ch}", name=f"fin{ch}") for ch in range(2)]
                fref_t = [rawf.tile([128, FREFW], F32, tag=f"fref{ch}", name=f"fref{ch}") for ch in range(2)]
                for ch in range(2):
                    for s0, sw in _subtiles(FINW):
                        nc.sync.dma_start(fin_t[ch][:, s0:s0 + sw],
                                          fin_d[ch * 128:(ch + 1) * 128, s0:s0 + sw])
                    for s0, sw in _subtiles(FREFW):
                        nc.sync.dma_start(fref_t[ch][:, s0:s0 + sw],
                                          fref_d[ch * 128:(ch + 1) * 128, s0:s0 + sw])

                _normalize(nc, tc, small, gpsum, fin_t, FINW, _subtiles(FINW), ones_col, ones_row)
                _normalize(nc, tc, small, gpsum, fref_t, FREFW, _subtiles(FREFW), ones_col, ones_row)

                for ch in range(2):
                    for f32t, h, lo, width in ((fin_t[ch], fin_h[ch], fin_l[ch], FINW),
                                               (fref_t[ch], fref_h[ch], fref_l[ch], FREFW)):
                        for s0, sw in _subtiles(width):
                            tmp = small.tile([128, NTW], F32, tag="sqtmp", name="spl")
                            nc.vector.tensor_copy(h[:, s0:s0 + sw], f32t[:, s0:s0 + sw])
                            nc.vector.tensor_copy(tmp[:, :sw], h[:, s0:s0 + sw])
                            nc.vector.tensor_sub(tmp[:, :sw], f32t[:, s0:s0 + sw], tmp[:, :sw])
                            nc.vector.tensor_copy(lo[:, s0:s0 + sw], tmp[:, :sw])

            corr_cm = tc.tile_pool(name="corr", bufs=2)
            corrp = corr_cm.__enter__()
            for mc in range(MC if not SKIP_GEMM else 0):
                m0 = mc * 128
                corr = corrp.tile([128, NREF], F32, tag="corr")
                for j in range(NT):
                    n0pix = j * 6 * W
                    pt = gpsum.tile([128, NTE], F32, tag="gemm")
                    n = 0
                    for fa, fb in ((fin_h, fref_h), (fin_h, fref_l), (fin_l, fref_h)):
                        for kt in range(18):
                            ch, dk = kt // 9, DKS[kt % 9]
                            n += 1
                            rhs = fb[ch][:, n0pix + dk:n0pix + dk + 6 * W] \
                                .rearrange("p (r c) -> p r c", c=W)[:, :, 0:W2]
                            nc.tensor.matmul(
                                pt[:],
                                fa[ch][:, m0 + dk:m0 + dk + 128],
                                rhs,
                                start=(n == 1), stop=(n == 54))
                    nc.vector.tensor_copy(corr[:, j * NTE:(j + 1) * NTE], pt[:])
                tv = small.tile([128, 8], F32, tag="tv")
                ti = small.tile([128, 8], U32, tag="ti")
                nc.vector.max(out=tv[:], in_=corr[:])
                nc.vector.max_index(out=ti[:], in_max=tv[:], in_values=corr[:])
                nc.sync.dma_start(topv_d[m0:m0 + 128, :], tv[:])
                nc.sync.dma_start(topi_d[m0:m0 + 128, :], ti[:])
            corr_cm.__exit__(None, None, None)

        # ---------------- phase 2: VGG head ----------------
        if SKIP_VGG:
            _ = 0
        with tc.tile_pool(name="vw", bufs=1) as vw, \
             tc.tile_pool(name="vbuf", bufs=1) as vbuf, \
             tc.tile_pool(name="vchunk", bufs=2) as vchunk, \
             tc.tile_pool(name="vpsum", bufs=4, space="PSUM") as vpsum:

            w1c_t = vw.tile([27, 64], F32)
            nc.sync.dma_start(w1c_t[:], w1c_d[:])
            w12p_t = vw.tile([128, 3, 64], F32)
            nc.sync.dma_start(w12p_t[:], w12p_d[:])
            w12l_t = vw.tile([64, 3, 64], F32)
            nc.sync.dma_start(w12l_t[:], w12l_d[:])
            w21p_t = vw.tile([128, 3, 128], F32)
            nc.sync.dma_start(w21p_t[:], w21p_d[:])
            w21l_t = vw.tile([64, 3, 128], F32)
            nc.sync.dma_start(w21l_t[:], w21l_d[:])
            w22_t = vw.tile([128, 9, 128], F32)
            nc.sync.dma_start(w22_t[:], w22_d[:])
            w31_t = vw.tile([128, 18, 128], F32)
            nc.sync.dma_start(w31_t[:], w31_d[:])
            b1_t = vw.tile([64, 1], F32)
            nc.sync.dma_start(b1_t[:], b1_d[:])
            b12_t = vw.tile([64, 1], F32)
            nc.sync.dma_start(b12_t[:], b12_d[:])
            b21_t = vw.tile([128, 1], F32)
            nc.sync.dma_start(b21_t[:], b21_d[:])
            b22_t = vw.tile([128, 1], F32)
            nc.sync.dma_start(b22_t[:], b22_d[:])
            b31_t = vw.tile([128, 2], F32)
            nc.sync.dma_start(b31_t[:], b31_d[:])
            vmask_t = vw.tile([128, JOBS, 132], F32)
            nc.sync.dma_start(vmask_t[:], vmask_d.rearrange("j p s -> p j s"))

            def stage_A(jj):
                """conv1_1 -> c11 (borders, mask, r1 out, pairing)."""
                c11 = vbuf.tile([128, C11R * W322], F32, tag="c11", name=f"c11_{jj}")
                nc.gpsimd.memset(c11[:], 0.0)
                nrows = 58
                r = 0
                while r < nrows:
                    cr = min(CR11, nrows - r)   # chunk c11 rows [1+r, 1+r+cr)
                    imc = vchunk.tile([27, CR11 * W322], F32, tag="vsA", name="imc")
                    for ki in range(3):
                        for kj in range(3):
                            p0 = (ki * 3 + kj) * 3
                            dst = imc[p0:p0 + 3, :cr * W322].rearrange(
                                "p (r x) -> p r x", x=W322)
                            nc.sync.dma_start(
                                dst, imgp_d[jj, :, r + ki:r + ki + cr, kj:kj + W322])
                    nflat = cr * W322
                    for t0, tw in _subtiles(nflat):
                        ps = vpsum.tile([64, NTW], F32, tag="vps", name="ps")
                        nc.tensor.matmul(ps[:, :tw], w1c_t[:], imc[:, t0:t0 + tw],
                                         start=True, stop=True)
                        nc.scalar.activation(
                            c11[0:64, (1 + r) * W322 + t0:(1 + r) * W322 + t0 + tw],
                            ps[:, :tw], AF.Relu, bias=b1_t[:])
                    r += cr
                # zero the wrap border pairs (x=321 of row r / x=0 of row r+1)
                c11v = c11[0:64].rearrange("p (r x) -> p r x", x=W322)
                nc.vector.memset(c11v[:, 0:C11R - 1, W322 - 1:W322], 0.0)
                nc.vector.memset(c11v[:, 1:C11R, 0:1], 0.0)
                # zero out-of-image halo rows (only boundary rows can be masked)
                for r0, r1 in ((0, 12), (48, 60)):
                    nc.vector.tensor_mul(
                        c11v[:, r0:r1, :], c11v[:, r0:r1, :],
                        vmask_t[0:64, jj, r0:r1].unsqueeze(2).broadcast_to((64, r1 - r0, W322)))
                # r1 out: rows 10..50, cols 1..321
                nc.sync.dma_start(r1_d[jj], c11v[:, 10:50, 1:321])
                # pairing: partitions 64:128 = col+1
                nc.vector.tensor_copy(c11[64:128, 0:C11R * W322 - 1],
                                      c11[0:64, 1:C11R * W322])
                nc.vector.memset(c11[64:128, C11R * W322 - 1:C11R * W322], 0.0)
                return c11

            def stage_B(jj, c11):
                """conv1_2 + pool1 -> p1 (double-buffered across jobs)."""
                p1 = vbuf.tile([128, P1R * P1W], F32, tag="p1", bufs=2, name=f"p1_{jj}")
                nc.gpsimd.memset(p1[:], 0.0)
                gr = 0
                pu = 1  # pool1 write row
                while gr < 56:
                    cr = min(CR12, 56 - gr)
                    cc = vchunk.tile([64, CR12 * W322], F32, tag="vsA", name="cc")
                    nflat = cr * W322
                    for t0, tw in _subtiles(nflat):
                        ps = vpsum.tile([64, NTW], F32, tag="vps", name="ps")
                        for ki in range(3):
                            base = (gr + 1 + ki) * W322 - 1 + t0
                            nc.tensor.matmul(ps[:, :tw], w12p_t[:, ki, :],
                                             c11[:, base:base + tw],
                                             start=(ki == 0), stop=False)
                            nc.tensor.matmul(ps[:, :tw], w12l_t[:, ki, :],
                                             c11[0:64, base + 2:base + 2 + tw],
                                             start=False, stop=(ki == 2))
                        nc.scalar.activation(cc[:, t0:t0 + tw], ps[:, :tw],
                                             AF.Relu, bias=b12_t[:])
                    # pool 2x2 -> p1 rows [pu, pu+cr/2), cols [1,161)
                    ccv2 = cc[:, :nflat].rearrange("p (r two x) -> p r two x",
                                                   two=2, x=W322)
                    rm = vchunk.tile([64, (CR12 // 2) * W322], F32, tag="vsB", bufs=1, name="rm")
                    rmv = rm[:, :(cr // 2) * W322].rearrange("p (r x) -> p r x", x=W322)
                    nc.vector.tensor_max(rmv[:], ccv2[:, :, 0, :], ccv2[:, :, 1, :])
                    rmp = rmv[:, :, 1:321].rearrange("p r (xh two) -> p r xh two", two=2)
                    p1v = p1[0:64].rearrange("p (r x) -> p r x", x=P1W)
                    nc.vector.tensor_max(p1v[:, pu:pu + cr // 2, 1:161],
                                         rmp[:, :, :, 0], rmp[:, :, :, 1])
                    gr += cr
                    pu += cr // 2
                # mask out-of-image pool1 rows, then pairing
                p1mv = p1[0:64].rearrange("p (r x) -> p r x", x=P1W)
                for r0, r1 in ((0, 6), (24, 30)):
                    nc.vector.tensor_mul(
                        p1mv[:, r0:r1, :], p1mv[:, r0:r1, :],
                        vmask_t[0:64, jj, 60 + r0:60 + r1].unsqueeze(2).broadcast_to((64, r1 - r0, P1W)))
                nc.vector.tensor_copy(p1[64:128, 0:P1R * P1W - 1],
                                      p1[0:64, 1:P1R * P1W])
                nc.vector.memset(p1[64:128, P1R * P1W - 1:P1R * P1W], 0.0)
                return p1

            def stage_C(jj, p1):
                """conv2_1 .. conv3_1 (+ r2/r3 outs)."""
                c21 = vbuf.tile([128, C21R * P1W], F32, tag="c21", name=f"c21_{jj}")
                c22 = vbuf.tile([128, C22R * P1W], F32, tag="c22", name=f"c22_{jj}")
                p2 = vbuf.tile([128, P2R * P2W], F32, tag="p2", name=f"p2_{jj}")
                c31 = vbuf.tile([128, 2, 10 * 80], F32, tag="c31", name=f"c31_{jj}")
                nc.gpsimd.memset(c21[:], 0.0)
                nc.gpsimd.memset(p2[:], 0.0)
                nflat = 26 * P1W
                for t0, tw in _subtiles(nflat):
                    ps = vpsum.tile([128, NTW], F32, tag="vps", name="ps")
                    for ki in range(3):
                        base = (1 + ki) * P1W - 1 + t0
                        nc.tensor.matmul(ps[:, :tw], w21p_t[:, ki, :],
                                         p1[:, base:base + tw],
                                         start=(ki == 0), stop=False)
                        nc.tensor.matmul(ps[:, :tw], w21l_t[:, ki, :],
                                         p1[0:64, base + 2:base + 2 + tw],
                                         start=False, stop=(ki == 2))
                    nc.scalar.activation(c21[:, P1W + t0:P1W + t0 + tw], ps[:, :tw],
                                         AF.Relu, bias=b21_t[:])
                c21v = c21[:].rearrange("p (r x) -> p r x", x=P1W)
                nc.vector.memset(c21v[:, 0:C21R - 1, P1W - 1:P1W], 0.0)
                nc.vector.memset(c21v[:, 1:C21R, 0:1], 0.0)
                for r0, r1 in ((0, 4), (24, 28)):
                    nc.vector.tensor_mul(
                        c21v[:, r0:r1, :], c21v[:, r0:r1, :],
                        vmask_t[:, jj, 90 + r0:90 + r1].unsqueeze(2).broadcast_to((128, r1 - r0, P1W)))
                # r2 out: c21 rows 4..24, cols 1..161
                nc.sync.dma_start(r2_d[jj], c21v[:, 4:24, 1:161])

                nflat = C22R * P1W
                for t0, tw in _subtiles(nflat):
                    ps = vpsum.tile([128, NTW], F32, tag="vps", name="ps")
                    for kt in range(9):
                        ki, kj = kt // 3, kt % 3
                        base = (1 + ki) * P1W - 1 + kj + t0
                        nc.tensor.matmul(ps[:, :tw], w22_t[:, kt, :],
                                         c21[:, base:base + tw],
                                         start=(kt == 0), stop=(kt == 8))
                    nc.scalar.activation(c22[:, t0:t0 + tw], ps[:, :tw],
                                         AF.Relu, bias=b22_t[:])
                # pool2 -> p2 rows [1,13), cols [1,81)
                c22v2 = c22[:].rearrange("p (r two x) -> p r two x", two=2, x=P1W)
                rm2 = vchunk.tile([128, 12 * P1W], F32, tag="vsB", bufs=1, name="rm2")
                rm2v = rm2[:].rearrange("p (r x) -> p r x", x=P1W)
                nc.vector.tensor_max(rm2v[:], c22v2[:, :, 0, :], c22v2[:, :, 1, :])
                rm2p = rm2v[:, :, 1:161].rearrange("p r (xh two) -> p r xh two", two=2)
                p2v = p2[:].rearrange("p (r x) -> p r x", x=P2W)
                nc.vector.tensor_max(p2v[:, 1:13, 1:81],
                                     rm2p[:, :, :, 0], rm2p[:, :, :, 1])
                for r0, r1 in ((0, 2), (12, 14)):
                    nc.vector.tensor_mul(
                        p2v[:, r0:r1, :], p2v[:, r0:r1, :],
                        vmask_t[:, jj, 118 + r0:118 + r1].unsqueeze(2).broadcast_to((128, r1 - r0, P2W)))

                for half in range(2):
                    for u0 in (0, 5):
                        ps = vpsum.tile([128, 5 * P2W], F32, tag="vps", name="ps")
                        for kt in range(9):
                            ki, kj = kt // 3, kt % 3
                            base = (u0 + 1 + ki) * P2W - 1 + kj
                            nc.tensor.matmul(ps[:], w31_t[:, 2 * kt + half, :],
                                             p2[:, base:base + 5 * P2W],
                                             start=(kt == 0), stop=(kt == 8))
                        psv = ps[:].rearrange("p (r x) -> p r x", x=P2W)
                        outv = c31[:, half].rearrange("p (r x) -> p r x", x=80)
                        nc.scalar.activation(outv[:, u0:u0 + 5, :],
                                             psv[:, :, 1:81], AF.Relu,
                                             bias=b31_t[:, half:half + 1])
                    nc.sync.dma_start(
                        r3_d[jj, half],
                        c31[:, half].rearrange("p (r x) -> p r x", x=80))

            if not SKIP_VGG:
                # software-pipeline the two jobs: job1 conv1_1 fills the PE
                # stalls in job0's pool/mask/pair tail, and job0's conv2+
                # stages overlap job1's conv1_1 eviction tail.
                c11_0 = stage_A(0)
                p1_0 = stage_B(0, c11_0)
                c11_1 = stage_A(1)
                stage_C(0, p1_0)
                p1_1 = stage_B(1, c11_1)
                stage_C(1, p1_1)

    return nc


# ---------------- host-side helpers ----------------
def _prep_inputs(dense_features1, dense_features2, img_ref_hr, vgg_params):
    d1 = np.ascontiguousarray(dense_features1, np.float32).reshape(B, C, PIX)
    d2 = np.ascontiguousarray(dense_features2, np.float32).reshape(B, C, PIX)
    img = np.ascontiguousarray(img_ref_hr, np.float32)

    mean = np.array([0.485, 0.456, 0.406], np.float32).reshape(3, 1, 1)
    std = np.array([0.229, 0.224, 0.225], np.float32).reshape(3, 1, 1)
    w11 = np.asarray(vgg_params["w1_1"], np.float32)
    b1f = np.asarray(vgg_params["b1_1"], np.float32)
    # image is normalized on host (padding must be zero in normalized domain)
    img = (img - mean[None]) / std[None]
    # layout [ (ki*3+kj)*3 + c, o ]
    w1c = np.ascontiguousarray(
        w11.transpose(2, 3, 1, 0).reshape(27, 64))

    def pair_weights(wkey, cin):
        wmat = np.asarray(vgg_params[wkey], np.float32)  # [o, i, 3, 3]
        cout = wmat.shape[0]
        wp = np.zeros((128, 3, cout), np.float32)
        wl = np.zeros((64, 3, cout), np.float32)
        for ki in range(3):
            wp[:cin, ki] = wmat[:, :, ki, 0].T
            wp[64:64 + cin, ki] = wmat[:, :, ki, 1].T
            wl[:cin, ki] = wmat[:, :, ki, 2].T
        return np.ascontiguousarray(wp), np.ascontiguousarray(wl)

    w12p, w12l = pair_weights("w1_2", 64)
    w21p, w21l = pair_weights("w2_1", 64)
    w22m = np.asarray(vgg_params["w2_2"], np.float32)
    w22 = np.ascontiguousarray(
        w22m.transpose(2, 3, 1, 0).reshape(9, 128, 128).transpose(1, 0, 2))
    w31m = np.asarray(vgg_params["w3_1"], np.float32)  # [256,128,3,3]
    w31 = np.zeros((128, 18, 128), np.float32)
    for kt in range(9):
        ki, kj = kt // 3, kt % 3
        w31[:, 2 * kt + 0] = w31m[0:128, :, ki, kj].T
        w31[:, 2 * kt + 1] = w31m[128:256, :, ki, kj].T

    common = {
        "w1c": w1c, "b1": b1f.reshape(64, 1).astype(np.float32),
        "w12p": w12p, "w12l": w12l,
        "b12": np.asarray(vgg_params["b1_2"], np.float32).reshape(64, 1),
        "w21p": w21p, "w21l": w21l,
        "b21": np.asarray(vgg_params["b2_1"], np.float32).reshape(128, 1),
        "w22": w22,
        "b22": np.asarray(vgg_params["b2_2"], np.float32).reshape(128, 1),
        "w31": np.ascontiguousarray(w31),
        "b31": np.ascontiguousarray(
            np.asarray(vgg_params["b3_1"], np.float32).reshape(2, 128).T),
    }

    in_maps = []
    for c in range(8):
        b, q = c // 4, c % 4
        fin = np.zeros((C, FINW), np.float32)
        lo = q * MPC
        hi = min(PIX, lo + FINW)
        fin[:, :hi - lo] = d1[b, :, lo:hi]
        fref = np.zeros((C, FREFW), np.float32)
        fref[:, :PIX] = d2[b]
        imgp = np.zeros((JOBS, 3, C11R, W328), np.float32)
        vmask = np.zeros((JOBS, 128, 132), np.float32)
        for j in range(JOBS):
            s = 2 * q + j
            g0 = 40 * s - 10       # imgp row 0 <-> image row g0
            r0 = max(0, -g0)
            r1 = min(C11R, 320 - g0)
            imgp[j, :, r0:r1, 2:322] = img[b, :, g0 + r0:g0 + r1, :]
            segs = [(0, C11R, 40 * s - 10, 320),    # c11: row lr <-> 40s-10+lr
                    (60, P1R, 20 * s - 5, 160),     # p1
                    (90, C21R, 20 * s - 4, 160),    # c21
                    (118, P2R, 10 * s - 2, 80)]     # p2
            for o0, n, gg0, lim in segs:
                rows = gg0 + np.arange(n)
                vmask[j, :, o0:o0 + n] = ((rows >= 0) & (rows < lim)).astype(np.float32)
        m = dict(common)
        m["fin"] = fin
        m["fref"] = fref
        m["imgp"] = imgp
        m["vmask"] = vmask
        in_maps.append(m)
    return in_maps


def _shift_np(x, si, sj):
    # shift content down/right with zero fill; x: [B,K,H,W,2]
    return np.pad(x, ((0, 0), (0, 0), (si, 0), (sj, 0), (0, 0)))[:, :, :x.shape[2], :x.shape[3], :]


def _postprocess(results):
    K = 3
    top_idx = np.zeros((B, K, W2, W2), np.int64)
    top_val = np.zeros((B, K, W2, W2), np.float32)
    m = np.arange(W2 * W)
    valid = (m % W) < W2
    scale = np.float32(1.0) / np.float32((3.0 + EPS_PATCH) * (3.0 + EPS_PATCH))
    for b in range(B):
        tv = np.concatenate([results[4 * b + q]["topv"] for q in range(4)], 0)[:W2 * W]
        ti = np.concatenate([results[4 * b + q]["topi"] for q in range(4)], 0)[:W2 * W]
        idx78 = ti[valid][:, :K].astype(np.int64)
        v = tv[valid][:, :K].astype(np.float32) * scale
        top_idx[b] = idx78.reshape(W2, W2, K).transpose(2, 0, 1)
        top_val[b] = v.reshape(W2, W2, K).transpose(2, 0, 1)

    fw = (top_idx % W2).astype(np.float32)
    fh = (top_idx // W2).astype(np.float32)
    gx = np.arange(W2, dtype=np.float32)
    gy = np.arange(W2, dtype=np.float32)
    flow = np.stack([fw - gx[None, None, None, :], fh - gy[None, None, :, None]], -1)
    flow3 = np.pad(flow, ((0, 0), (0, 0), (0, 2), (0, 2), (0, 0)))
    off3 = np.stack([_shift_np(flow3, i, j) for i in range(3) for j in range(3)], 2)
    sim3 = np.pad(top_val, ((0, 0), (0, 0), (1, 1), (1, 1)))

    flow2 = np.repeat(np.repeat(flow3, 2, 2), 2, 3) * np.float32(2.0)
    off2 = np.stack([_shift_np(flow2, 2 * i, 2 * j) for i in range(3) for j in range(3)], 2)
    sim2 = np.repeat(np.repeat(sim3, 2, 2), 2, 3)

    flow1 = np.repeat(np.repeat(flow3, 4, 2), 4, 3) * np.float32(4.0)
    off1 = np.stack([_shift_np(flow1, 4 * i, 4 * j) for i in range(3) for j in range(3)], 2)
    sim1 = np.repeat(np.repeat(sim3, 4, 2), 4, 3)

    r1 = np.zeros((B, 64, 320, 320), np.float32)
    r2 = np.zeros((B, 128, 160, 160), np.float32)
    r3 = np.zeros((B, 256, 80, 80), np.float32)
    for c in range(8):
        b, q = c // 4, c % 4
        for j in range(JOBS):
            s = 2 * q + j
            r1[b, :, 40 * s:40 * s + 40, :] = results[c]["r1o"][j]
            r2[b, :, 20 * s:20 * s + 20, :] = results[c]["r2o"][j]
            r3[b, 0:128, 10 * s:10 * s + 10, :] = results[c]["r3o"][j, 0]
            r3[b, 128:256, 10 * s:10 * s + 10, :] = results[c]["r3o"][j, 1]

    return (flow1.astype(np.float32), flow2.astype(np.float32), flow3.astype(np.float32),
            off1.astype(np.float32), off2.astype(np.float32), off3.astype(np.float32),
            sim1, sim2, sim3, r1, r2, r3)


LAST_RUN_SECONDS = None


def kernel(dense_features1, dense_features2, img_ref_hr, vgg_params):
    import time
    global LAST_RUN_SECONDS
    if "nc" not in _CACHE:
        nc = _build_program()
        _split_excess_waits(nc)  # hardware codegen: <=1 sync wait per inst
        _CACHE["nc"] = nc
    nc = _CACHE["nc"]
    in_maps = _prep_inputs(dense_features1, dense_features2, img_ref_hr, vgg_params)
    t0 = time.time()
    res = run_bass_kernel_spmd(nc, in_maps, list(range(8)))
    LAST_RUN_SECONDS = time.time() - t0
    return _postprocess(res.results)
